# revision 2
# baseline (speedup 1.0000x reference)
"""Trainium2 Bass kernel for nn_MultiHeadAttention (B=1, S=4096, D=2048, H=16, HD=128).

Sharding: tensor-parallel over heads — 2 heads per core on 8 NeuronCores.
Each core computes its 2 heads' Q/K/V projections, causal attention, and a
partial output projection (row-split Wo); the host sums the 8 partials and
adds the output bias (the all-reduce/unshard step).

Key structural idea vs the earlier revision: the softmax denominator used to
cost a dedicated ones-column matmul per p-tile — as many PE moving cycles as
the attn@V matmul itself (~61us/core). This version computes attention in the
[q, d] orientation instead: p is the STATIONARY operand (128-q chunks) and V,
augmented with a literal ones column, is the MOVING operand. One matmul then
yields psO[q, 0:128] = p^T V and psO[q, 128] = sum_k p (the denominator) —
the denominator is free (+1 moving cycle per 128). q-blocks are 256 wide so
the four live [q, d+1] accumulators (2 heads x 2 q-chunks) plus the 2-deep
scores ring plus a 2-slot scratch ring fit the 8 PSUM banks exactly (PSUM
slots are bank-granular).

Layout/schedule (per core, matmuls bf16 with fp32 PSUM):
  - X^T streamed in eight 512-col slices (double-buffered); slice 0 and the
    V weights arrive as small leading chunks in separate tiles so the first
    V matmuls chase the DMA stream (sync-engine descriptor issue rate is the
    startup bottleneck). Projections are FUSED into attention: slice sl's Q
    is emitted first, then its V/K units interleave into attention blocks
    2sl/2sl+1 as PE fill-work while ACT catches up on the exp queue (K/V
    land before the diagonal pair that needs them). Interleaved K bias adds
    go on DVE so they do not delay the exp stream on ACT.
  - Q, K produced transposed [d, s]; scores computed transposed per k-tile
    pair into one PSUM bank, one wide exp per pair. Causal masking is
    multiplicative post-exp; the diagonal pair is narrowed triangularly
    (tile i only covers q >= 128*i) and the AV chunk matmuls narrow the
    same way for free.
  - attn@V: stationary = p [128k, 128q chunk], moving = [V_h | 1] [128k,
    129]; accumulated over all k-tiles into psO[h][qc]. Normalize =
    reciprocal of the denominator column times the 128 value columns (DVE,
    per-partition scalar), then a PE transpose (identity permutation)
    restores outt to [d, s] bf16 for the O-projection. No partition
    broadcasts, no denominator folds.
  - O-projection unchanged: out[s, e] += outt_h^T @ WoT_h accumulated over
    both heads; per s-tile the 4 PSUM results gather into one [128, 2048]
    SBUF tile, stored with a single DMA. PSUM->SBUF copies alternate
    DVE/ACT.

Build notes:
  - Bacc (not raw Bass): walrus encodes at most ONE sem wait per
    instruction; Bacc's generate_event_semaphores pass splits larger sets.
  - PSUM banks: scores ring 2 (bufs=2 x [128,512]f32) + psO 4 (bufs=4) +
    scratch ring 2 (bufs=2: proj psq/psv, O-proj psF, transpose psT) = 8.
"""

import numpy as np
import ml_dtypes

import concourse.bass as bass
import concourse.mybir as mybir
import concourse.tile as tile
from concourse import bacc
from concourse.bass_utils import run_bass_kernel_spmd


S = 4096          # sequence length
D = 2048          # model dim
NCORES = 8
DL = D // NCORES  # 256 local head dims (2 heads)
NH = 2            # heads per core
HD = 128          # head dim
QB = 256          # q block width
NQB = S // QB     # 16
KT = 128          # k tile (partitions)
NKT = S // KT     # 32
ET = 128          # e contraction tile
NET = D // ET     # 16
NST = S // 128    # 32 s-tiles
VW = 2 * (HD + 1)  # vt cols per s-tile: [h0 d0..127, 1 | h1 d0..127, 1]
SQ = 512          # X^T streaming slice width (s columns)
NSQ = S // SQ     # 8 slices
SCALE = 1.0 / np.sqrt(HD)

BF16 = mybir.dt.bfloat16
F32 = mybir.dt.float32


def build_nc(is_causal: bool) -> bass.Bass:
    nc = bacc.Bacc()

    # xt2 row-block sl: [128, et*512+c] = X[sl*512+c, et*128+p] (host packed)
    XT2 = nc.dram_tensor("xt2", [NSQ * 128, NET * SQ], BF16, kind="ExternalInput")
    # weights packed [128, et*256+c] = W^T[et*128+p, c]
    WQ2 = nc.dram_tensor("wq2", [128, NET * DL], BF16, kind="ExternalInput")
    WK2 = nc.dram_tensor("wk2", [128, NET * DL], BF16, kind="ExternalInput")
    WV2 = nc.dram_tensor("wv2", [128, NET * DL], BF16, kind="ExternalInput")
    # bias columns [128, 4]: bq.d0 | bq.d1 | bk.d0 | bk.d1
    BQKC = nc.dram_tensor("bqkc", [128, 4], F32, kind="ExternalInput")
    BVROW = nc.dram_tensor("bvrow", [1, DL], BF16, kind="ExternalInput")
    # [128, h*2048+c] = Wo^T[h*128+p, c]
    WO2 = nc.dram_tensor("wo2", [128, NH * D], BF16, kind="ExternalInput")
    # [128, 256] multiplicative causal mask: m[k, c] = (k <= c)
    MASKS2 = nc.dram_tensor("masks2", [128, QB], BF16, kind="ExternalInput")
    IDT = nc.dram_tensor("idt", [128, 128], BF16, kind="ExternalInput")
    OUT = nc.dram_tensor("out", [S, D], F32, kind="ExternalOutput")

    with tile.TileContext(nc) as tc:
        with tc.tile_pool(name="persist", bufs=1) as persist:
            # Q head0 | Q head1 | K head0 | K head1, each [128, 4096]
            qkt = persist.tile([128, 4 * S], BF16, name="qkt")
            # V with interleaved ones cols: s-tile st at [st*VW, (st+1)*VW),
            # head h at +h*129; col +h*129+128 stays 1.0 (memset below)
            vt = persist.tile([128, NST * VW], BF16, name="vt")
            wot_sb = persist.tile([128, NH * D], BF16, name="wot_sb")
            masks_sb = persist.tile([128, QB], BF16, name="masks_sb")
            wk_sb = persist.tile([128, NET * DL], BF16, name="wk_sb")
            wq_sb = persist.tile([128, NET * DL], BF16, name="wq_sb")
            idt_sb = persist.tile([128, 128], BF16, name="idt_sb")
            biasqk = persist.tile([128, 4], F32, name="biasqk")
            bvrow_sb = persist.tile([1, DL], BF16, name="bvrow_sb")
            bvb_sb = persist.tile([128, DL], BF16, name="bvb_sb")
            # normalized attention outputs, transposed: (h*NQB+qb) tile [128d, 256q]
            outt = persist.tile([128, NH * NQB * QB], BF16, name="outt")

            # pre-set the interleaved ones columns (V writes overwrite d-cols)
            nc.vector.memset(vt[:, :], 1.0)

            with tc.tile_pool(name="xtp", bufs=2) as xtp, \
                 tc.tile_pool(name="scor", bufs=2, space="PSUM") as scor, \
                 tc.tile_pool(name="pso", bufs=4, space="PSUM") as pso, \
                 tc.tile_pool(name="pa", bufs=2, space="PSUM") as pa, \
                 tc.tile_pool(name="pp", bufs=8) as pp, \
                 tc.tile_pool(name="rp", bufs=4) as rp, \
                 tc.tile_pool(name="op", bufs=2) as op:

                # DMA order: V weights + X^T slice 0, chunked and interleaved
                # so the first V matmuls start after ~1 MB; then the rest.
                xt_tiles = {}
                def load_xe(sl):
                    xt_e = xtp.tile([128, NET * SQ], BF16, name="xt_e", tag="xt")
                    xt_tiles[sl] = xt_e
                    nc.sync.dma_start(
                        out=xt_e[:, :], in_=XT2[sl * 128 : (sl + 1) * 128, :]
                    )
                XCHUNKS = [2, 2, 4, 4, 4]           # et tiles per chunk
                XOFF = [0, 2, 4, 8, 12]             # et offset per chunk
                def chunk_of(et):
                    for ci in range(len(XCHUNKS) - 1, -1, -1):
                        if et >= XOFF[ci]:
                            return ci, et - XOFF[ci]
                xt0c = [
                    persist.tile([128, n * SQ], BF16, name=f"xt0c{c}")
                    for c, n in enumerate(XCHUNKS)
                ]
                wv_cs = [
                    persist.tile([128, n * DL], BF16, name=f"wv_c{c}")
                    for c, n in enumerate(XCHUNKS)
                ]
                for ci, n in enumerate(XCHUNKS):
                    nc.sync.dma_start(
                        out=wv_cs[ci][:, :],
                        in_=WV2[:, XOFF[ci] * DL : (XOFF[ci] + n) * DL],
                    )
                    nc.sync.dma_start(
                        out=xt0c[ci][:, :],
                        in_=XT2[0:128, XOFF[ci] * SQ : (XOFF[ci] + n) * SQ],
                    )
                nc.sync.dma_start(out=biasqk[:, :], in_=BQKC[:, :])
                nc.sync.dma_start(out=wk_sb[:, :], in_=WK2[:, :])
                if is_causal:
                    nc.sync.dma_start(out=masks_sb[:, :], in_=MASKS2[:, :])
                nc.sync.dma_start(out=wq_sb[:, :], in_=WQ2[:, :])
                nc.sync.dma_start(out=bvrow_sb[:, :], in_=BVROW[:, :])
                nc.sync.dma_start(out=idt_sb[:, :], in_=IDT[:, :])
                nc.sync.dma_start(out=wot_sb[:, :], in_=WO2[:, :])
                # broadcast bv across partitions once; folded into each V
                # tile's PSUM->SBUF copy below
                nc.gpsimd.partition_broadcast(bvb_sb[:, :], bvrow_sb[:, :])

                def store_v(psv, st):
                    # psv [128, 256] f32 -> vt d-cols (ones cols untouched)
                    for h in range(NH):
                        nc.vector.scalar_tensor_tensor(
                            out=vt[:, st * VW + h * 129 : st * VW + h * 129 + 128],
                            in0=psv[:, h * 128 : (h + 1) * 128],
                            scalar=1.0,
                            in1=bvb_sb[:, h * 128 : (h + 1) * 128],
                            op0=mybir.AluOpType.mult,
                            op1=mybir.AluOpType.add,
                        )

                def emit_v_tile0(stl):
                    psv = scor.tile([128, SQ], F32, name="psv0", tag="sc")
                    for et in range(NET):
                        ci, le = chunk_of(et)
                        nc.tensor.matmul(
                            psv[:, :DL],
                            lhsT=xt0c[ci][:, le * SQ + stl * 128 : le * SQ + (stl + 1) * 128],
                            rhs=wv_cs[ci][:, le * DL : (le + 1) * DL],
                            start=(et == 0),
                            stop=(et == NET - 1),
                        )
                    store_v(psv[:, :DL], stl)

                def emit_qk0(w_sb, base4, bias_base, dt):
                    psq = scor.tile([128, SQ], F32, name="psq0", tag="sc")
                    for et in range(NET):
                        ci, le = chunk_of(et)
                        nc.tensor.matmul(
                            psq[:, :SQ],
                            lhsT=w_sb[:, et * DL + dt * 128 : et * DL + (dt + 1) * 128],
                            rhs=xt0c[ci][:, le * SQ : (le + 1) * SQ],
                            start=(et == 0),
                            stop=(et == NET - 1),
                        )
                    nc.scalar.add(
                        qkt[:, (base4 + dt) * S : (base4 + dt) * S + SQ],
                        psq[:, :SQ],
                        biasqk[:, bias_base + dt : bias_base + dt + 1],
                    )

                def emit_v_tile(sl, stl):
                    xt_e = xt_tiles[sl]
                    st = sl * (SQ // 128) + stl
                    psv = pa.tile([128, DL], F32, name="psv", tag="pa")
                    for et in range(NET):
                        ci, le = chunk_of(et)
                        nc.tensor.matmul(
                            psv[:, :DL],
                            lhsT=xt_e[:, et * SQ + stl * 128 : et * SQ + (stl + 1) * 128],
                            rhs=wv_cs[ci][:, le * DL : (le + 1) * DL],
                            start=(et == 0),
                            stop=(et == NET - 1),
                        )
                    store_v(psv[:, :DL], st)

                def emit_qk(sl, w_sb, base4, bias_base, dt, on_dve=False):
                    # transposed [d, s] projection for one head. Bias add on
                    # ACT normally; interleaved K units use DVE so they do
                    # not delay the exp stream queued on ACT.
                    xt_e = xt_tiles[sl]
                    psq = pa.tile([128, SQ], F32, name="psq", tag="pa")
                    for et in range(NET):
                        nc.tensor.matmul(
                            psq[:, :],
                            lhsT=w_sb[:, et * DL + dt * 128 : et * DL + (dt + 1) * 128],
                            rhs=xt_e[:, et * SQ : (et + 1) * SQ],
                            start=(et == 0),
                            stop=(et == NET - 1),
                        )
                    dst = qkt[:, (base4 + dt) * S + sl * SQ : (base4 + dt) * S + (sl + 1) * SQ]
                    if on_dve:
                        nc.vector.tensor_scalar_add(
                            out=dst, in0=psq[:, :],
                            scalar1=biasqk[:, bias_base + dt : bias_base + dt + 1],
                        )
                    else:
                        nc.scalar.add(
                            dst, psq[:, :],
                            biasqk[:, bias_base + dt : bias_base + dt + 1],
                        )

                def vslice(kt, h):
                    return vt[:, kt * VW + h * 129 : kt * VW + (h + 1) * 129]

                def emit_norm_chain(psO, qb, h, qc, on_act):
                    # 1/denom (col 128) times the value cols, then a PE
                    # transpose back to [d, q] for the O-projection
                    recip = rp.tile([128, 1], F32, name="recip", tag="recip")
                    nc.vector.reciprocal_approx_fast(
                        recip[:, :], psO[h, qc][:, 128:129]
                    )
                    stg = rp.tile([128, 128], BF16, name="stg", tag="stg")
                    nc.vector.tensor_scalar_mul(
                        out=stg[:, :], in0=psO[h, qc][:, 0:128],
                        scalar1=recip[:, :],
                    )
                    psT = pa.tile([128, 128], BF16, name="psT", tag="pa")
                    nc.tensor.transpose(psT[:, :], stg[:, :], idt_sb[:, :])
                    dst = outt[:, (h * NQB + qb) * QB + qc * 128 :
                               (h * NQB + qb) * QB + (qc + 1) * 128]
                    if on_act:
                        nc.scalar.copy(dst, psT[:, :])
                    else:
                        nc.vector.tensor_copy(dst, psT[:, :])

                def o_proj(qb):
                    for j in range(2):
                        st = qb * 2 + j
                        osb = op.tile([128, D], F32, name="osb", tag="osb")
                        for et in range(4):
                            psF = pa.tile([128, 512], F32, name="psF", tag="pa")
                            for h in range(NH):
                                o_base = (h * NQB + qb) * QB + j * 128
                                nc.tensor.matmul(
                                    psF[:, :],
                                    lhsT=outt[:, o_base : o_base + 128],
                                    rhs=wot_sb[:, h * D + et * 512 : h * D + (et + 1) * 512],
                                    start=(h == 0),
                                    stop=(h == NH - 1),
                                )
                            if et % 2 == 0:
                                nc.vector.tensor_copy(
                                    osb[:, et * 512 : (et + 1) * 512], psF[:, :]
                                )
                            else:
                                nc.scalar.copy(
                                    osb[:, et * 512 : (et + 1) * 512], psF[:, :]
                                )
                        nc.sync.dma_start(
                            out=OUT[st * 128 : (st + 1) * 128, :], in_=osb[:, :]
                        )

                def attention_qb(qb, units=None):
                    npairs = (qb + 1) if is_causal else NQB
                    units = list(units) if units else []
                    per_gap = -(-len(units) // max(1, npairs - 1)) if units else 0
                    psO = {}
                    for h in range(NH):
                        for qc in range(2):
                            psO[h, qc] = pso.tile(
                                [128, 512], F32, name="psO", tag="o"
                            )
                    for pi in range(npairs):
                        diag = is_causal and pi == qb
                        if units and diag:
                            while units:
                                units.pop(0)()
                        elif units and pi > 0:
                            for _ in range(per_gap):
                                if units:
                                    units.pop(0)()
                        for h in range(NH):
                            qb0 = h * S + qb * QB
                            psS = scor.tile([128, SQ], F32, name="psS", tag="sc")
                            p2 = pp.tile([128, SQ], BF16, name="p2", tag="p")
                            if not diag:
                                for half in range(2):
                                    kt = 2 * pi + half
                                    nc.tensor.matmul(
                                        psS[:, half * QB : (half + 1) * QB],
                                        lhsT=qkt[:, (2 + h) * S + kt * 128 : (2 + h) * S + (kt + 1) * 128],
                                        rhs=qkt[:, qb0 : qb0 + QB],
                                        start=True,
                                        stop=True,
                                    )
                                nc.scalar.activation(
                                    p2[:, :], psS[:, :],
                                    mybir.ActivationFunctionType.Exp,
                                    scale=float(SCALE),
                                )
                                last = (not is_causal) and pi == npairs - 1
                                for half in range(2):
                                    kt = 2 * pi + half
                                    for qc in range(2):
                                        nc.tensor.matmul(
                                            psO[h, qc][:, :129],
                                            lhsT=p2[:, half * QB + qc * 128 : half * QB + (qc + 1) * 128],
                                            rhs=vslice(kt, h),
                                            start=(pi == 0 and half == 0),
                                            stop=(last and half == 1),
                                        )
                            else:
                                # diagonal pair: tile i=0 spans the full 256
                                # (masked), tile i=1 only q-cols 128:256
                                kt0, kt1 = 2 * qb, 2 * qb + 1
                                first = qb == 0
                                nc.tensor.matmul(
                                    psS[:, 0:QB],
                                    lhsT=qkt[:, (2 + h) * S + kt0 * 128 : (2 + h) * S + (kt0 + 1) * 128],
                                    rhs=qkt[:, qb0 : qb0 + QB],
                                    start=True,
                                    stop=True,
                                )
                                nc.tensor.matmul(
                                    psS[:, QB + 128 : SQ],
                                    lhsT=qkt[:, (2 + h) * S + kt1 * 128 : (2 + h) * S + (kt1 + 1) * 128],
                                    rhs=qkt[:, qb0 + 128 : qb0 + QB],
                                    start=True,
                                    stop=True,
                                )
                                nc.scalar.activation(
                                    p2[:, 0:QB], psS[:, 0:QB],
                                    mybir.ActivationFunctionType.Exp,
                                    scale=float(SCALE),
                                )
                                nc.scalar.activation(
                                    p2[:, QB + 128 : SQ], psS[:, QB + 128 : SQ],
                                    mybir.ActivationFunctionType.Exp,
                                    scale=float(SCALE),
                                )
                                nc.vector.tensor_mul(
                                    p2[:, 0:QB], p2[:, 0:QB], masks_sb[:, 0:QB]
                                )
                                nc.vector.tensor_mul(
                                    p2[:, QB + 128 : SQ], p2[:, QB + 128 : SQ],
                                    masks_sb[:, 0:128],
                                )
                                nc.tensor.matmul(
                                    psO[h, 0][:, :129],
                                    lhsT=p2[:, 0:128],
                                    rhs=vslice(kt0, h),
                                    start=first,
                                    stop=True,
                                )
                                nc.tensor.matmul(
                                    psO[h, 1][:, :129],
                                    lhsT=p2[:, 128:256],
                                    rhs=vslice(kt0, h),
                                    start=first,
                                    stop=False,
                                )
                                nc.tensor.matmul(
                                    psO[h, 1][:, :129],
                                    lhsT=p2[:, QB + 128 : SQ],
                                    rhs=vslice(kt1, h),
                                    start=False,
                                    stop=True,
                                )
                                # emit this head's normalize+transpose now so
                                # it overlaps the other head's diagonal
                                for qc in range(2):
                                    emit_norm_chain(psO, qb, h, qc, on_act=(qc == 1))
                    if not is_causal:
                        for h in range(NH):
                            for qc in range(2):
                                emit_norm_chain(psO, qb, h, qc, on_act=(qc == 1))
                    o_proj(qb)

                if is_causal:
                    for sl in range(NSQ):
                        if sl + 1 < NSQ:
                            load_xe(sl + 1)
                        if sl == 0:
                            for j in range(SQ // 128):
                                emit_v_tile0(j)
                            for d in range(NH):
                                emit_qk0(wk_sb, 2, 2, d)
                            for d in range(NH):
                                emit_qk0(wq_sb, 0, 0, d)
                            attention_qb(0, [])
                            attention_qb(1, [])
                            continue
                        for dt in range(NH):
                            emit_qk(sl, wq_sb, 0, 0, dt)
                        units_a = [
                            (lambda s=sl, j=j: emit_v_tile(s, j)) for j in (0, 1)
                        ] + [
                            (lambda s=sl, d=d: emit_qk(s, wk_sb, 2, 2, d, on_dve=True))
                            for d in range(NH)
                        ]
                        units_b = [
                            (lambda s=sl, j=j: emit_v_tile(s, j)) for j in (2, 3)
                        ]
                        attention_qb(2 * sl, units_a)
                        attention_qb(2 * sl + 1, units_b)
                else:
                    for sl in range(NSQ):
                        if sl + 1 < NSQ:
                            load_xe(sl + 1)
                        if sl == 0:
                            for j in range(SQ // 128):
                                emit_v_tile0(j)
                            for d in range(NH):
                                emit_qk0(wk_sb, 2, 2, d)
                            for d in range(NH):
                                emit_qk0(wq_sb, 0, 0, d)
                            continue
                        for j in range(SQ // 128):
                            emit_v_tile(sl, j)
                        for w_sb, base4, bias_base in ((wq_sb, 0, 0), (wk_sb, 2, 2)):
                            for dt in range(NH):
                                emit_qk(sl, w_sb, base4, bias_base, dt)
                    for qb in range(NQB):
                        attention_qb(qb)
    nc.finalize()
    return nc


def _bf16(a: np.ndarray) -> np.ndarray:
    return np.ascontiguousarray(a.astype(ml_dtypes.bfloat16))


def make_in_maps(X, Wq, bq, Wk, bk, Wv, bv, Wo, is_causal: bool):
    x2d = np.asarray(X, dtype=np.float32).reshape(S, D)
    # xt2[sl*128+p, et*512+c] = X^T[et*128+p, sl*512+c]
    xt2 = _bf16(
        x2d.T.reshape(NET, 128, NSQ, SQ)
        .transpose(2, 1, 0, 3)
        .reshape(NSQ * 128, NET * SQ)
    )
    ki = np.arange(128)[:, None]
    qj = np.arange(QB)[None, :]
    masks = (ki <= qj).astype(ml_dtypes.bfloat16)
    idt = np.eye(128, dtype=ml_dtypes.bfloat16)

    def _pack_w(wT):  # [D, DL] -> [128, NET*DL]
        return _bf16(
            np.ascontiguousarray(wT).reshape(NET, 128, DL)
            .transpose(1, 0, 2)
            .reshape(128, NET * DL)
        )

    in_maps = []
    for c in range(NCORES):
        sl = slice(c * DL, (c + 1) * DL)
        wot = np.asarray(Wo)[:, sl].T  # [DL, D]
        wo2 = _bf16(wot.reshape(NH, 128, D).transpose(1, 0, 2).reshape(128, NH * D))
        in_maps.append(
            {
                "xt2": xt2,
                "wq2": _pack_w(np.asarray(Wq)[sl, :].T),
                "wk2": _pack_w(np.asarray(Wk)[sl, :].T),
                "wv2": _pack_w(np.asarray(Wv)[sl, :].T),
                "bqkc": np.ascontiguousarray(
                    np.stack(
                        [
                            np.asarray(bq, dtype=np.float32)[sl][:128],
                            np.asarray(bq, dtype=np.float32)[sl][128:],
                            np.asarray(bk, dtype=np.float32)[sl][:128],
                            np.asarray(bk, dtype=np.float32)[sl][128:],
                        ],
                        axis=1,
                    )
                ),
                "bvrow": _bf16(np.asarray(bv)[None, sl]),
                "wo2": wo2,
                "masks2": masks,
                "idt": idt,
            }
        )
    return in_maps


_NC_CACHE: dict = {}


def _get_nc(is_causal: bool) -> bass.Bass:
    if is_causal not in _NC_CACHE:
        _NC_CACHE[is_causal] = build_nc(is_causal)
    return _NC_CACHE[is_causal]


def kernel(X, Wq, bq, Wk, bk, Wv, bv, Wo, bo, is_causal, **run_kwargs):
    causal = bool(int(np.asarray(is_causal)))
    nc = _get_nc(causal)
    in_maps = make_in_maps(X, Wq, bq, Wk, bk, Wv, bv, Wo, causal)
    res = run_bass_kernel_spmd(nc, in_maps, core_ids=list(range(NCORES)), **run_kwargs)
    out = np.asarray(bo, dtype=np.float32)[None, :].repeat(S, axis=0)
    for c in range(NCORES):
        out += res.results[c]["out"]
    return out.reshape(1, S, D)


# revision 6
# speedup vs baseline: 1.0802x; 1.0802x over previous
"""Trainium2 Bass kernel for nn_MultiHeadAttention (B=1, S=4096, D=2048, H=16, HD=128).

Sharding: tensor-parallel over heads — 2 heads per core on 8 NeuronCores.
Each core computes its 2 heads' Q/K/V projections, causal attention, and a
partial output projection (row-split Wo); the host sums the 8 partials and
adds the output bias (the all-reduce/unshard step).

Key structural idea vs the earlier revision: the softmax denominator used to
cost a dedicated ones-column matmul per p-tile — as many PE moving cycles as
the attn@V matmul itself (~61us/core). This version computes attention in the
[q, d] orientation instead: p is the STATIONARY operand (128-q chunks) and V,
augmented with a literal ones column, is the MOVING operand. One matmul then
yields psO[q, 0:128] = p^T V and psO[q, 128] = sum_k p (the denominator) —
the denominator is free (+1 moving cycle per 128). q-blocks are 256 wide so
the four live [q, d+1] accumulators (2 heads x 2 q-chunks) plus the 2-deep
scores ring plus a 2-slot scratch ring fit the 8 PSUM banks exactly (PSUM
slots are bank-granular).

Layout/schedule (per core, matmuls bf16 with fp32 PSUM):
  - X^T streamed in eight 512-col slices (double-buffered); slice 0 and the
    V weights arrive as small leading chunks in separate tiles so the first
    V matmuls chase the DMA stream (sync-engine descriptor issue rate is the
    startup bottleneck). Projections are FUSED into attention: slice sl's Q
    is emitted first, then its V/K units interleave into attention blocks
    2sl/2sl+1 as PE fill-work while ACT catches up on the exp queue (K/V
    land before the diagonal pair that needs them). Interleaved K bias adds
    go on DVE so they do not delay the exp stream on ACT.
  - Q, K produced transposed [d, s]; scores computed transposed per k-tile
    pair into one PSUM bank, one wide exp per pair. Causal masking is
    multiplicative post-exp; the diagonal pair is narrowed triangularly
    (tile i only covers q >= 128*i) and the AV chunk matmuls narrow the
    same way for free.
  - attn@V: stationary = p [128k, 128q chunk], moving = [V_h | 1] [128k,
    129]; accumulated over all k-tiles into psO[h][qc]. Normalize =
    reciprocal of the denominator column times the 128 value columns (DVE,
    per-partition scalar), then a PE transpose (identity permutation)
    restores outt to [d, s] bf16 for the O-projection. No partition
    broadcasts, no denominator folds.
  - O-projection unchanged: out[s, e] += outt_h^T @ WoT_h accumulated over
    both heads; per s-tile the 4 PSUM results gather into one [128, 2048]
    SBUF tile, stored with a single DMA. PSUM->SBUF copies alternate
    DVE/ACT.

Build notes:
  - Bacc (not raw Bass): walrus encodes at most ONE sem wait per
    instruction; Bacc's generate_event_semaphores pass splits larger sets.
  - PSUM banks: scores ring 2 (bufs=2 x [128,512]f32) + psO 4 (bufs=4) +
    scratch ring 2 (bufs=2: proj psq/psv, O-proj psF, transpose psT) = 8.
"""

import numpy as np
import ml_dtypes

import concourse.bass as bass
import concourse.mybir as mybir
import concourse.tile as tile
from concourse import bacc
from concourse.bass_utils import run_bass_kernel_spmd


S = 4096          # sequence length
D = 2048          # model dim
NCORES = 8
DL = D // NCORES  # 256 local head dims (2 heads)
NH = 2            # heads per core
HD = 128          # head dim
QB = 256          # q block width
NQB = S // QB     # 16
KT = 128          # k tile (partitions)
NKT = S // KT     # 32
ET = 128          # e contraction tile
NET = D // ET     # 16
NST = S // 128    # 32 s-tiles
VW = 2 * (HD + 1)  # vt cols per s-tile: [h0 d0..127, 1 | h1 d0..127, 1]
SQ = 512          # X^T streaming slice width (s columns)
NSQ = S // SQ     # 8 slices
SCALE = 1.0 / np.sqrt(HD)

BF16 = mybir.dt.bfloat16
F32 = mybir.dt.float32


def build_nc(is_causal: bool) -> bass.Bass:
    nc = bacc.Bacc()

    # xt2 row-block sl: [128, et*512+c] = X[sl*512+c, et*128+p] (host packed)
    XT2 = nc.dram_tensor("xt2", [NSQ * 128, NET * SQ], BF16, kind="ExternalInput")
    # weights packed [128, et*256+c] = W^T[et*128+p, c]
    WQ2 = nc.dram_tensor("wq2", [128, NET * DL], BF16, kind="ExternalInput")
    WK2 = nc.dram_tensor("wk2", [128, NET * DL], BF16, kind="ExternalInput")
    WV2 = nc.dram_tensor("wv2", [128, NET * DL], BF16, kind="ExternalInput")
    # bias columns [128, 4]: bq.d0 | bq.d1 | bk.d0 | bk.d1
    BQKC = nc.dram_tensor("bqkc", [128, 4], F32, kind="ExternalInput")
    BVROW = nc.dram_tensor("bvrow", [1, DL], BF16, kind="ExternalInput")
    # [128, h*2048+c] = Wo^T[h*128+p, c]
    WO2 = nc.dram_tensor("wo2", [128, NH * D], BF16, kind="ExternalInput")
    # [128, 256] multiplicative causal mask: m[k, c] = (k <= c)
    MASKS2 = nc.dram_tensor("masks2", [128, QB], BF16, kind="ExternalInput")
    IDT = nc.dram_tensor("idt", [128, 128], BF16, kind="ExternalInput")
    OUT = nc.dram_tensor("out", [S, D], F32, kind="ExternalOutput")

    with tile.TileContext(nc) as tc:
        with tc.tile_pool(name="persist", bufs=1) as persist:
            # Q head0 | Q head1 | K head0 | K head1, each [128, 4096]
            qkt = persist.tile([128, 4 * S], BF16, name="qkt")
            # V with interleaved ones cols: s-tile st at [st*VW, (st+1)*VW),
            # head h at +h*129; col +h*129+128 stays 1.0 (memset below)
            vt = persist.tile([128, NST * VW], BF16, name="vt")
            wot_sb = persist.tile([128, NH * D], BF16, name="wot_sb")
            masks_sb = persist.tile([128, QB], BF16, name="masks_sb")
            wk_sb = persist.tile([128, NET * DL], BF16, name="wk_sb")
            wq_sb = persist.tile([128, NET * DL], BF16, name="wq_sb")
            idt_sb = persist.tile([128, 128], BF16, name="idt_sb")
            biasqk = persist.tile([128, 4], F32, name="biasqk")
            bvrow_sb = persist.tile([1, DL], BF16, name="bvrow_sb")
            bvb_sb = persist.tile([128, DL], BF16, name="bvb_sb")
            # normalized attention outputs, transposed: (h*NQB+qb) tile [128d, 256q]
            outt = persist.tile([128, NH * NQB * QB], BF16, name="outt")

            # pre-set the interleaved ones columns (V writes overwrite d-cols)
            nc.vector.memset(vt[:, :], 1.0)

            with tc.tile_pool(name="xtp", bufs=2) as xtp, \
                 tc.tile_pool(name="scor", bufs=2, space="PSUM") as scor, \
                 tc.tile_pool(name="pso", bufs=4, space="PSUM") as pso, \
                 tc.tile_pool(name="pa", bufs=2, space="PSUM") as pa, \
                 tc.tile_pool(name="pp", bufs=8) as pp, \
                 tc.tile_pool(name="rp", bufs=4) as rp, \
                 tc.tile_pool(name="op", bufs=2) as op:

                # DMA order: V weights + X^T slice 0, chunked and interleaved
                # so the first V matmuls start after ~1 MB; then the rest.
                xt_tiles = {}
                def load_xe(sl):
                    xt_e = xtp.tile([128, NET * SQ], BF16, name="xt_e", tag="xt")
                    xt_tiles[sl] = xt_e
                    nc.sync.dma_start(
                        out=xt_e[:, :], in_=XT2[sl * 128 : (sl + 1) * 128, :]
                    )
                XCHUNKS = [2, 2, 4, 4, 4]           # et tiles per chunk
                XOFF = [0, 2, 4, 8, 12]             # et offset per chunk
                def chunk_of(et):
                    for ci in range(len(XCHUNKS) - 1, -1, -1):
                        if et >= XOFF[ci]:
                            return ci, et - XOFF[ci]
                xt0c = [
                    persist.tile([128, n * SQ], BF16, name=f"xt0c{c}")
                    for c, n in enumerate(XCHUNKS)
                ]
                wv_cs = [
                    persist.tile([128, n * DL], BF16, name=f"wv_c{c}")
                    for c, n in enumerate(XCHUNKS)
                ]
                for ci, n in enumerate(XCHUNKS):
                    nc.sync.dma_start(
                        out=wv_cs[ci][:, :],
                        in_=WV2[:, XOFF[ci] * DL : (XOFF[ci] + n) * DL],
                    )
                    nc.sync.dma_start(
                        out=xt0c[ci][:, :],
                        in_=XT2[0:128, XOFF[ci] * SQ : (XOFF[ci] + n) * SQ],
                    )
                nc.sync.dma_start(out=biasqk[:, :], in_=BQKC[:, :])
                # wk/wq in halves: subtile deps let the first 8 et-matmuls
                # start after half the transfer
                HNW = NET * DL // 2
                nc.sync.dma_start(out=wk_sb[:, :HNW], in_=WK2[:, :HNW])
                nc.sync.dma_start(out=wk_sb[:, HNW:], in_=WK2[:, HNW:])
                nc.sync.dma_start(out=wq_sb[:, :HNW], in_=WQ2[:, :HNW])
                nc.sync.dma_start(out=wq_sb[:, HNW:], in_=WQ2[:, HNW:])
                if is_causal:
                    nc.sync.dma_start(out=masks_sb[:, :], in_=MASKS2[:, :])
                nc.sync.dma_start(out=bvrow_sb[:, :], in_=BVROW[:, :])
                nc.sync.dma_start(out=idt_sb[:, :], in_=IDT[:, :])
                nc.sync.dma_start(out=wot_sb[:, :], in_=WO2[:, :])
                # broadcast bv across partitions once; folded into each V
                # tile's PSUM->SBUF copy below
                nc.gpsimd.partition_broadcast(bvb_sb[:, :], bvrow_sb[:, :])

                def store_v(psv, st):
                    # psv [128, 256] f32 -> vt d-cols (ones cols untouched)
                    for h in range(NH):
                        nc.vector.scalar_tensor_tensor(
                            out=vt[:, st * VW + h * 129 : st * VW + h * 129 + 128],
                            in0=psv[:, h * 128 : (h + 1) * 128],
                            scalar=1.0,
                            in1=bvb_sb[:, h * 128 : (h + 1) * 128],
                            op0=mybir.AluOpType.mult,
                            op1=mybir.AluOpType.add,
                        )

                def emit_v_tile0(stl):
                    psv = scor.tile([128, SQ], F32, name="psv0", tag="sc")
                    for et in range(NET):
                        ci, le = chunk_of(et)
                        nc.tensor.matmul(
                            psv[:, :DL],
                            lhsT=xt0c[ci][:, le * SQ + stl * 128 : le * SQ + (stl + 1) * 128],
                            rhs=wv_cs[ci][:, le * DL : (le + 1) * DL],
                            start=(et == 0),
                            stop=(et == NET - 1),
                        )
                    store_v(psv[:, :DL], stl)

                def emit_qk0(w_sb, base4, bias_base, dt):
                    psq = scor.tile([128, SQ], F32, name="psq0", tag="sc")
                    for et in range(NET):
                        ci, le = chunk_of(et)
                        nc.tensor.matmul(
                            psq[:, :SQ],
                            lhsT=w_sb[:, et * DL + dt * 128 : et * DL + (dt + 1) * 128],
                            rhs=xt0c[ci][:, le * SQ : (le + 1) * SQ],
                            start=(et == 0),
                            stop=(et == NET - 1),
                        )
                    nc.scalar.add(
                        qkt[:, (base4 + dt) * S : (base4 + dt) * S + SQ],
                        psq[:, :SQ],
                        biasqk[:, bias_base + dt : bias_base + dt + 1],
                    )

                def emit_v_tile(sl, stl):
                    xt_e = xt_tiles[sl]
                    st = sl * (SQ // 128) + stl
                    psv = pa.tile([128, DL], F32, name="psv", tag="pa")
                    for et in range(NET):
                        ci, le = chunk_of(et)
                        nc.tensor.matmul(
                            psv[:, :DL],
                            lhsT=xt_e[:, et * SQ + stl * 128 : et * SQ + (stl + 1) * 128],
                            rhs=wv_cs[ci][:, le * DL : (le + 1) * DL],
                            start=(et == 0),
                            stop=(et == NET - 1),
                        )
                    store_v(psv[:, :DL], st)

                def emit_qk(sl, w_sb, base4, bias_base, dt, on_dve=False):
                    # transposed [d, s] projection for one head. Bias add on
                    # ACT normally; interleaved K units use DVE so they do
                    # not delay the exp stream queued on ACT.
                    xt_e = xt_tiles[sl]
                    psq = pa.tile([128, SQ], F32, name="psq", tag="pa")
                    for et in range(NET):
                        nc.tensor.matmul(
                            psq[:, :],
                            lhsT=w_sb[:, et * DL + dt * 128 : et * DL + (dt + 1) * 128],
                            rhs=xt_e[:, et * SQ : (et + 1) * SQ],
                            start=(et == 0),
                            stop=(et == NET - 1),
                        )
                    dst = qkt[:, (base4 + dt) * S + sl * SQ : (base4 + dt) * S + (sl + 1) * SQ]
                    if on_dve:
                        nc.vector.tensor_scalar_add(
                            out=dst, in0=psq[:, :],
                            scalar1=biasqk[:, bias_base + dt : bias_base + dt + 1],
                        )
                    else:
                        nc.scalar.add(
                            dst, psq[:, :],
                            biasqk[:, bias_base + dt : bias_base + dt + 1],
                        )

                def vslice(kt, h):
                    return vt[:, kt * VW + h * 129 : kt * VW + (h + 1) * 129]

                def emit_norm_chain(psO, qb, h, qc, on_act):
                    # 1/denom (col 128) times the value cols, then a PE
                    # transpose back to [d, q] for the O-projection
                    recip = rp.tile([128, 1], F32, name="recip", tag="recip")
                    nc.vector.reciprocal_approx_fast(
                        recip[:, :], psO[h, qc][:, 128:129]
                    )
                    stg = rp.tile([128, 128], BF16, name="stg", tag="stg")
                    nc.vector.tensor_scalar_mul(
                        out=stg[:, :], in0=psO[h, qc][:, 0:128],
                        scalar1=recip[:, :],
                    )
                    psT = pa.tile([128, 128], BF16, name="psT", tag="pa")
                    nc.tensor.transpose(psT[:, :], stg[:, :], idt_sb[:, :])
                    dst = outt[:, (h * NQB + qb) * QB + qc * 128 :
                               (h * NQB + qb) * QB + (qc + 1) * 128]
                    if on_act:
                        nc.scalar.copy(dst, psT[:, :])
                    else:
                        nc.vector.tensor_copy(dst, psT[:, :])

                def o_proj(qb, tail=False):
                    for j in range(2):
                        st = qb * 2 + j
                        osb = op.tile([128, D], F32, name="osb", tag="osb")
                        for et in range(4):
                            psF = pa.tile([128, 512], F32, name="psF", tag="pa")
                            for h in range(NH):
                                o_base = (h * NQB + qb) * QB + j * 128
                                nc.tensor.matmul(
                                    psF[:, :],
                                    lhsT=outt[:, o_base : o_base + 128],
                                    rhs=wot_sb[:, h * D + et * 512 : h * D + (et + 1) * 512],
                                    start=(h == 0),
                                    stop=(h == NH - 1),
                                )
                            if tail and et % 2 == 1:
                                # tail: ACT is idle — split the drain copies
                                nc.scalar.copy(
                                    osb[:, et * 512 : (et + 1) * 512], psF[:, :]
                                )
                            else:
                                nc.vector.tensor_copy(
                                    osb[:, et * 512 : (et + 1) * 512], psF[:, :]
                                )
                            if tail and et == 1:
                                nc.sync.dma_start(
                                    out=OUT[st * 128 : (st + 1) * 128, :1024],
                                    in_=osb[:, :1024],
                                )
                        if tail:
                            nc.sync.dma_start(
                                out=OUT[st * 128 : (st + 1) * 128, 1024:],
                                in_=osb[:, 1024:],
                            )
                        else:
                            nc.sync.dma_start(
                                out=OUT[st * 128 : (st + 1) * 128, :], in_=osb[:, :]
                            )

                def attention_qb(qb, units=None):
                    # Software-pipelined: AV matmuls run one pair behind the
                    # scores/exp stream (so PE never waits on the exp it just
                    # queued), and the PREVIOUS block's O-projection is
                    # emitted after pair 0 as block-boundary fill.
                    npairs = (qb + 1) if is_causal else NQB
                    units = list(units) if units else []
                    per_gap = -(-len(units) // max(1, npairs - 1)) if units else 0
                    psO = {}
                    for h in range(NH):
                        for qc in range(2):
                            psO[h, qc] = pso.tile(
                                [128, 512], F32, name="psO", tag="o"
                            )

                    def emit_scores(pi, h):
                        qb0 = h * S + qb * QB
                        psS = scor.tile([128, SQ], F32, name="psS", tag="sc")
                        p2 = pp.tile([128, SQ], BF16, name="p2", tag="p")
                        if is_causal and pi == qb:
                            # diagonal pair: tile i=0 spans the full 256
                            # (masked), tile i=1 only q-cols 128:256; one exp
                            kt0, kt1 = 2 * qb, 2 * qb + 1
                            nc.tensor.matmul(
                                psS[:, 0:QB],
                                lhsT=qkt[:, (2 + h) * S + kt0 * 128 : (2 + h) * S + (kt0 + 1) * 128],
                                rhs=qkt[:, qb0 : qb0 + QB],
                                start=True,
                                stop=True,
                            )
                            nc.tensor.matmul(
                                psS[:, QB : QB + 128],
                                lhsT=qkt[:, (2 + h) * S + kt1 * 128 : (2 + h) * S + (kt1 + 1) * 128],
                                rhs=qkt[:, qb0 + 128 : qb0 + QB],
                                start=True,
                                stop=True,
                            )
                            nc.scalar.activation(
                                p2[:, 0 : QB + 128], psS[:, 0 : QB + 128],
                                mybir.ActivationFunctionType.Exp,
                                scale=float(SCALE),
                            )
                            nc.vector.tensor_mul(
                                p2[:, 0:QB], p2[:, 0:QB], masks_sb[:, 0:QB]
                            )
                            nc.vector.tensor_mul(
                                p2[:, QB : QB + 128], p2[:, QB : QB + 128],
                                masks_sb[:, 0:128],
                            )
                        else:
                            for half in range(2):
                                kt = 2 * pi + half
                                nc.tensor.matmul(
                                    psS[:, half * QB : (half + 1) * QB],
                                    lhsT=qkt[:, (2 + h) * S + kt * 128 : (2 + h) * S + (kt + 1) * 128],
                                    rhs=qkt[:, qb0 : qb0 + QB],
                                    start=True,
                                    stop=True,
                                )
                            nc.scalar.activation(
                                p2[:, :], psS[:, :],
                                mybir.ActivationFunctionType.Exp,
                                scale=float(SCALE),
                            )
                        return p2

                    def emit_av(pi, h, p2):
                        if is_causal and pi == qb:
                            kt0, kt1 = 2 * qb, 2 * qb + 1
                            first = qb == 0
                            nc.tensor.matmul(
                                psO[h, 0][:, :129],
                                lhsT=p2[:, 0:128],
                                rhs=vslice(kt0, h),
                                start=first,
                                stop=True,
                            )
                            nc.tensor.matmul(
                                psO[h, 1][:, :129],
                                lhsT=p2[:, 128:256],
                                rhs=vslice(kt0, h),
                                start=first,
                                stop=False,
                            )
                            nc.tensor.matmul(
                                psO[h, 1][:, :129],
                                lhsT=p2[:, QB : QB + 128],
                                rhs=vslice(kt1, h),
                                start=False,
                                stop=True,
                            )
                        else:
                            last = (not is_causal) and pi == npairs - 1
                            for half in range(2):
                                kt = 2 * pi + half
                                for qc in range(2):
                                    nc.tensor.matmul(
                                        psO[h, qc][:, :129],
                                        lhsT=p2[:, half * QB + qc * 128 : half * QB + (qc + 1) * 128],
                                        rhs=vslice(kt, h),
                                        start=(pi == 0 and half == 0),
                                        stop=(last and half == 1),
                                    )

                    prev = None  # (pi, p2_h0, p2_h1)
                    for pi in range(npairs):
                        diag = is_causal and pi == qb
                        if units and diag:
                            while units:
                                units.pop(0)()
                        elif units and pi > 0:
                            for _ in range(per_gap):
                                if units:
                                    units.pop(0)()
                        p2s = [emit_scores(pi, h) for h in range(NH)]
                        if pi == 0 and qb >= 1:
                            o_proj(qb - 1)
                        if prev is not None:
                            for h in range(NH):
                                emit_av(prev[0], h, prev[1 + h])
                        if diag:
                            for h in range(NH):
                                emit_av(pi, h, p2s[h])
                                for qc in range(2):
                                    emit_norm_chain(psO, qb, h, qc, on_act=False)
                            prev = None
                        else:
                            prev = (pi, p2s[0], p2s[1])
                    if prev is not None:  # non-causal: drain last pair
                        for h in range(NH):
                            emit_av(prev[0], h, prev[1 + h])
                    if not is_causal:
                        for h in range(NH):
                            for qc in range(2):
                                emit_norm_chain(psO, qb, h, qc, on_act=False)
                        o_proj(qb)

                if is_causal:
                    for sl in range(NSQ):
                        if sl == 0:
                            for j in range(SQ // 128):
                                emit_v_tile0(j)
                            for d in range(NH):
                                emit_qk0(wk_sb, 2, 2, d)
                            for d in range(NH):
                                emit_qk0(wq_sb, 0, 0, d)
                            # slice-1 X load AFTER the startup burst so its
                            # 2MB transfer does not delay the weight DMAs
                            load_xe(1)
                            attention_qb(0, [])
                            attention_qb(1, [])
                            continue
                        if sl + 1 < NSQ:
                            load_xe(sl + 1)
                        for dt in range(NH):
                            emit_qk(sl, wq_sb, 0, 0, dt)
                        units_a = [
                            (lambda s=sl, j=j: emit_v_tile(s, j)) for j in (0, 1)
                        ] + [
                            (lambda s=sl, d=d: emit_qk(s, wk_sb, 2, 2, d, on_dve=True))
                            for d in range(NH)
                        ]
                        units_b = [
                            (lambda s=sl, j=j: emit_v_tile(s, j)) for j in (2, 3)
                        ]
                        attention_qb(2 * sl, units_a)
                        attention_qb(2 * sl + 1, units_b)
                    o_proj(NQB - 1, tail=True)
                else:
                    for sl in range(NSQ):
                        if sl + 1 < NSQ:
                            load_xe(sl + 1)
                        if sl == 0:
                            for j in range(SQ // 128):
                                emit_v_tile0(j)
                            for d in range(NH):
                                emit_qk0(wk_sb, 2, 2, d)
                            for d in range(NH):
                                emit_qk0(wq_sb, 0, 0, d)
                            continue
                        for j in range(SQ // 128):
                            emit_v_tile(sl, j)
                        for w_sb, base4, bias_base in ((wq_sb, 0, 0), (wk_sb, 2, 2)):
                            for dt in range(NH):
                                emit_qk(sl, w_sb, base4, bias_base, dt)
                    for qb in range(NQB):
                        attention_qb(qb)
    nc.finalize()
    return nc


def _bf16(a: np.ndarray) -> np.ndarray:
    return np.ascontiguousarray(a.astype(ml_dtypes.bfloat16))


def make_in_maps(X, Wq, bq, Wk, bk, Wv, bv, Wo, is_causal: bool):
    x2d = np.asarray(X, dtype=np.float32).reshape(S, D)
    # xt2[sl*128+p, et*512+c] = X^T[et*128+p, sl*512+c]
    xt2 = _bf16(
        x2d.T.reshape(NET, 128, NSQ, SQ)
        .transpose(2, 1, 0, 3)
        .reshape(NSQ * 128, NET * SQ)
    )
    ki = np.arange(128)[:, None]
    qj = np.arange(QB)[None, :]
    masks = (ki <= qj).astype(ml_dtypes.bfloat16)
    idt = np.eye(128, dtype=ml_dtypes.bfloat16)

    def _pack_w(wT):  # [D, DL] -> [128, NET*DL]
        return _bf16(
            np.ascontiguousarray(wT).reshape(NET, 128, DL)
            .transpose(1, 0, 2)
            .reshape(128, NET * DL)
        )

    in_maps = []
    for c in range(NCORES):
        sl = slice(c * DL, (c + 1) * DL)
        wot = np.asarray(Wo)[:, sl].T  # [DL, D]
        wo2 = _bf16(wot.reshape(NH, 128, D).transpose(1, 0, 2).reshape(128, NH * D))
        in_maps.append(
            {
                "xt2": xt2,
                "wq2": _pack_w(np.asarray(Wq)[sl, :].T),
                "wk2": _pack_w(np.asarray(Wk)[sl, :].T),
                "wv2": _pack_w(np.asarray(Wv)[sl, :].T),
                "bqkc": np.ascontiguousarray(
                    np.stack(
                        [
                            np.asarray(bq, dtype=np.float32)[sl][:128],
                            np.asarray(bq, dtype=np.float32)[sl][128:],
                            np.asarray(bk, dtype=np.float32)[sl][:128],
                            np.asarray(bk, dtype=np.float32)[sl][128:],
                        ],
                        axis=1,
                    )
                ),
                "bvrow": _bf16(np.asarray(bv)[None, sl]),
                "wo2": wo2,
                "masks2": masks,
                "idt": idt,
            }
        )
    return in_maps


_NC_CACHE: dict = {}


def _get_nc(is_causal: bool) -> bass.Bass:
    if is_causal not in _NC_CACHE:
        _NC_CACHE[is_causal] = build_nc(is_causal)
    return _NC_CACHE[is_causal]


def kernel(X, Wq, bq, Wk, bk, Wv, bv, Wo, bo, is_causal, **run_kwargs):
    causal = bool(int(np.asarray(is_causal)))
    nc = _get_nc(causal)
    in_maps = make_in_maps(X, Wq, bq, Wk, bk, Wv, bv, Wo, causal)
    res = run_bass_kernel_spmd(nc, in_maps, core_ids=list(range(NCORES)), **run_kwargs)
    out = np.asarray(bo, dtype=np.float32)[None, :].repeat(S, axis=0)
    for c in range(NCORES):
        out += res.results[c]["out"]
    return out.reshape(1, S, D)


# revision 12
# speedup vs baseline: 1.2125x; 1.1224x over previous
"""Trainium2 Bass kernel for nn_MultiHeadAttention (B=1, S=4096, D=2048, H=16, HD=128).

Sharding: tensor-parallel over heads — 2 heads per core on 8 NeuronCores.
Each core computes its 2 heads' Q/K/V projections, causal attention, and a
partial output projection (row-split Wo); the host sums the 8 partials and
adds the output bias (the all-reduce/unshard step).

Key structural idea vs the earlier revision: the softmax denominator used to
cost a dedicated ones-column matmul per p-tile — as many PE moving cycles as
the attn@V matmul itself (~61us/core). This version computes attention in the
[q, d] orientation instead: p is the STATIONARY operand (128-q chunks) and V,
augmented with a literal ones column, is the MOVING operand. One matmul then
yields psO[q, 0:128] = p^T V and psO[q, 128] = sum_k p (the denominator) —
the denominator is free (+1 moving cycle per 128). q-blocks are 256 wide so
the four live [q, d+1] accumulators (2 heads x 2 q-chunks) plus the 2-deep
scores ring plus a 2-slot scratch ring fit the 8 PSUM banks exactly (PSUM
slots are bank-granular).

Layout/schedule (per core, matmuls bf16 with fp32 PSUM):
  - X^T streamed in eight 512-col slices (double-buffered); slice 0 and the
    V weights arrive as small leading chunks in separate tiles so the first
    V matmuls chase the DMA stream (sync-engine descriptor issue rate is the
    startup bottleneck). Projections are FUSED into attention: slice sl's Q
    is emitted first, then its V/K units interleave into attention blocks
    2sl/2sl+1 as PE fill-work while ACT catches up on the exp queue (K/V
    land before the diagonal pair that needs them). Interleaved K bias adds
    go on DVE so they do not delay the exp stream on ACT.
  - Q, K produced transposed [d, s]; scores computed transposed per k-tile
    pair into one PSUM bank, one wide exp per pair. Causal masking is
    multiplicative post-exp; the diagonal pair is narrowed triangularly
    (tile i only covers q >= 128*i) and the AV chunk matmuls narrow the
    same way for free.
  - attn@V: stationary = p [128k, 128q chunk], moving = [V_h | 1] [128k,
    129]; accumulated over all k-tiles into psO[h][qc]. Normalize =
    reciprocal of the denominator column times the 128 value columns (DVE,
    per-partition scalar), then a PE transpose (identity permutation)
    restores outt to [d, s] bf16 for the O-projection. No partition
    broadcasts, no denominator folds.
  - O-projection unchanged: out[s, e] += outt_h^T @ WoT_h accumulated over
    both heads; per s-tile the 4 PSUM results gather into one [128, 2048]
    SBUF tile, stored with a single DMA. PSUM->SBUF copies alternate
    DVE/ACT.

Build notes:
  - Bacc (not raw Bass): walrus encodes at most ONE sem wait per
    instruction; Bacc's generate_event_semaphores pass splits larger sets.
  - PSUM banks: scores ring 2 (bufs=2 x [128,512]f32) + psO 4 (bufs=4) +
    scratch ring 2 (bufs=2: proj psq/psv, O-proj psF, transpose psT) = 8.
"""

import numpy as np
import ml_dtypes

import concourse.bass as bass
import concourse.mybir as mybir
import concourse.tile as tile
from concourse import bacc
from concourse.bass_utils import run_bass_kernel_spmd


S = 4096          # sequence length
D = 2048          # model dim
NCORES = 8
DL = D // NCORES  # 256 local head dims (2 heads)
NH = 2            # heads per core
HD = 128          # head dim
QB = 256          # q block width
NQB = S // QB     # 16
KT = 128          # k tile (partitions)
NKT = S // KT     # 32
ET = 128          # e contraction tile
NET = D // ET     # 16
NST = S // 128    # 32 s-tiles
VW = 2 * (HD + 1)  # vt cols per s-tile: [h0 d0..127, 1 | h1 d0..127, 1]
SQ = 512          # X^T streaming slice width (s columns)
NSQ = S // SQ     # 8 slices
SCALE = 1.0 / np.sqrt(HD)

BF16 = mybir.dt.bfloat16
F32 = mybir.dt.float32


def build_nc(is_causal: bool) -> bass.Bass:
    nc = bacc.Bacc()

    # xt2 row-block sl: [128, et*512+c] = X[sl*512+c, et*128+p] (host packed)
    XT2 = nc.dram_tensor("xt2", [NSQ * 128, NET * SQ], BF16, kind="ExternalInput")
    # weights packed [128, et*256+c] = W^T[et*128+p, c]
    WQ2 = nc.dram_tensor("wq2", [128, NET * DL], BF16, kind="ExternalInput")
    WK2 = nc.dram_tensor("wk2", [128, NET * DL], BF16, kind="ExternalInput")
    WV2 = nc.dram_tensor("wv2", [128, NET * DL], BF16, kind="ExternalInput")
    # bias columns [128, 4]: bq.d0 | bq.d1 | bk.d0 | bk.d1
    BQKC = nc.dram_tensor("bqkc", [128, 4], F32, kind="ExternalInput")
    BVROW = nc.dram_tensor("bvrow", [1, DL], BF16, kind="ExternalInput")
    # [128, h*2048+c] = Wo^T[h*128+p, c]
    WO2 = nc.dram_tensor("wo2", [128, NH * D], BF16, kind="ExternalInput")
    # [128, 256] multiplicative causal mask: m[k, c] = (k <= c)
    MASKS2 = nc.dram_tensor("masks2", [128, QB], BF16, kind="ExternalInput")
    IDT = nc.dram_tensor("idt", [128, 128], BF16, kind="ExternalInput")
    OUT = nc.dram_tensor("out", [S, D], F32, kind="ExternalOutput")

    with tile.TileContext(nc) as tc:
        with tc.tile_pool(name="persist", bufs=1) as persist:
            # Q head0 | Q head1 | K head0 | K head1, each [128, 4096]
            qkt = persist.tile([128, 4 * S], BF16, name="qkt")
            # V with interleaved ones cols: s-tile st at [st*VW, (st+1)*VW),
            # head h at +h*129; col +h*129+128 stays 1.0 (memset below)
            vt = persist.tile([128, NST * VW], BF16, name="vt")
            wot_sb = persist.tile([128, NH * D], BF16, name="wot_sb")
            masks_sb = persist.tile([128, QB], BF16, name="masks_sb")
            wk_sb = persist.tile([128, NET * DL], BF16, name="wk_sb")
            wq_sb = persist.tile([128, NET * DL], BF16, name="wq_sb")
            idt_sb = persist.tile([128, 128], BF16, name="idt_sb")
            biasqk = persist.tile([128, 4], F32, name="biasqk")
            bvrow_sb = persist.tile([1, DL], BF16, name="bvrow_sb")
            bvb_sb = persist.tile([128, DL], BF16, name="bvb_sb")
            # normalized attention outputs, transposed: (h*NQB+qb) tile [128d, 256q]
            outt = persist.tile([128, NH * NQB * QB], BF16, name="outt")

            # pre-set the interleaved ones columns (V writes overwrite d-cols)
            nc.vector.memset(vt[:, :], 1.0)

            with tc.tile_pool(name="xtp", bufs=2) as xtp, \
                 tc.tile_pool(name="scor", bufs=2, space="PSUM") as scor, \
                 tc.tile_pool(name="pso", bufs=4, space="PSUM") as pso, \
                 tc.tile_pool(name="pa", bufs=2, space="PSUM") as pa, \
                 tc.tile_pool(name="pp", bufs=8) as pp, \
                 tc.tile_pool(name="rp", bufs=4) as rp, \
                 tc.tile_pool(name="op", bufs=2) as op:

                # DMA order: V weights + X^T slice 0, chunked and interleaved
                # so the first V matmuls start after ~1 MB; then the rest.
                xt_tiles = {}
                def load_xe(sl):
                    xt_e = xtp.tile([128, NET * SQ], BF16, name="xt_e", tag="xt")
                    xt_tiles[sl] = xt_e
                    nc.sync.dma_start(
                        out=xt_e[:, :], in_=XT2[sl * 128 : (sl + 1) * 128, :]
                    )
                XCHUNKS = [2, 2, 4, 4, 4]           # et tiles per chunk
                XOFF = [0, 2, 4, 8, 12]             # et offset per chunk
                def chunk_of(et):
                    for ci in range(len(XCHUNKS) - 1, -1, -1):
                        if et >= XOFF[ci]:
                            return ci, et - XOFF[ci]
                xt0c = [
                    persist.tile([128, n * SQ], BF16, name=f"xt0c{c}")
                    for c, n in enumerate(XCHUNKS)
                ]
                wv_cs = [
                    persist.tile([128, n * DL], BF16, name=f"wv_c{c}")
                    for c, n in enumerate(XCHUNKS)
                ]
                # wk/wq in halves (subtile deps let the first 8 et-matmuls
                # start early), interleaved into the chunk stream by the
                # PE-time each transfer is needed
                HNW = NET * DL // 2
                def chunk_dma(ci):
                    n = XCHUNKS[ci]
                    nc.sync.dma_start(
                        out=wv_cs[ci][:, :],
                        in_=WV2[:, XOFF[ci] * DL : (XOFF[ci] + n) * DL],
                    )
                    nc.sync.dma_start(
                        out=xt0c[ci][:, :],
                        in_=XT2[0:128, XOFF[ci] * SQ : (XOFF[ci] + n) * SQ],
                    )
                chunk_dma(0)
                chunk_dma(1)
                chunk_dma(2)
                nc.sync.dma_start(out=wk_sb[:, :HNW], in_=WK2[:, :HNW])
                chunk_dma(3)
                nc.sync.dma_start(out=wk_sb[:, HNW:], in_=WK2[:, HNW:])
                nc.sync.dma_start(out=wq_sb[:, :HNW], in_=WQ2[:, :HNW])
                chunk_dma(4)
                nc.sync.dma_start(out=wq_sb[:, HNW:], in_=WQ2[:, HNW:])
                nc.sync.dma_start(out=biasqk[:, :], in_=BQKC[:, :])
                if is_causal:
                    nc.sync.dma_start(out=masks_sb[:, :], in_=MASKS2[:, :])
                nc.sync.dma_start(out=bvrow_sb[:, :], in_=BVROW[:, :])
                nc.sync.dma_start(out=idt_sb[:, :], in_=IDT[:, :])
                nc.sync.dma_start(out=wot_sb[:, :], in_=WO2[:, :])
                # broadcast bv across partitions once; folded into each V
                # tile's PSUM->SBUF copy below
                nc.gpsimd.partition_broadcast(bvb_sb[:, :], bvrow_sb[:, :])

                def store_v(psv, st):
                    # psv [128, 256] f32 -> vt d-cols (ones cols untouched)
                    for h in range(NH):
                        nc.vector.scalar_tensor_tensor(
                            out=vt[:, st * VW + h * 129 : st * VW + h * 129 + 128],
                            in0=psv[:, h * 128 : (h + 1) * 128],
                            scalar=1.0,
                            in1=bvb_sb[:, h * 128 : (h + 1) * 128],
                            op0=mybir.AluOpType.mult,
                            op1=mybir.AluOpType.add,
                        )

                def emit_v0_all():
                    # et-major over all 4 s-tiles so each DMA chunk is
                    # consumed as late as possible (pso ring is free here)
                    psvs = [
                        pso.tile([128, 512], F32, name="psv0", tag="o")
                        for _ in range(SQ // 128)
                    ]
                    for et in range(NET):
                        ci, le = chunk_of(et)
                        for stl in range(SQ // 128):
                            nc.tensor.matmul(
                                psvs[stl][:, :DL],
                                lhsT=xt0c[ci][:, le * SQ + stl * 128 : le * SQ + (stl + 1) * 128],
                                rhs=wv_cs[ci][:, le * DL : (le + 1) * DL],
                                start=(et == 0),
                                stop=(et == NET - 1),
                            )
                    for stl in range(SQ // 128):
                        store_v(psvs[stl][:, :DL], stl)

                def emit_qk0(w_sb, base4, bias_base, dt):
                    psq = scor.tile([128, SQ], F32, name="psq0", tag="sc")
                    for et in range(NET):
                        ci, le = chunk_of(et)
                        nc.tensor.matmul(
                            psq[:, :SQ],
                            lhsT=w_sb[:, et * DL + dt * 128 : et * DL + (dt + 1) * 128],
                            rhs=xt0c[ci][:, le * SQ : (le + 1) * SQ],
                            start=(et == 0),
                            stop=(et == NET - 1),
                        )
                    nc.scalar.add(
                        qkt[:, (base4 + dt) * S : (base4 + dt) * S + SQ],
                        psq[:, :SQ],
                        biasqk[:, bias_base + dt : bias_base + dt + 1],
                    )

                def emit_v_tile(sl, stl):
                    xt_e = xt_tiles[sl]
                    st = sl * (SQ // 128) + stl
                    psv = pa.tile([128, DL], F32, name="psv", tag="pa")
                    for et in range(NET):
                        ci, le = chunk_of(et)
                        nc.tensor.matmul(
                            psv[:, :DL],
                            lhsT=xt_e[:, et * SQ + stl * 128 : et * SQ + (stl + 1) * 128],
                            rhs=wv_cs[ci][:, le * DL : (le + 1) * DL],
                            start=(et == 0),
                            stop=(et == NET - 1),
                        )
                    store_v(psv[:, :DL], st)

                def emit_qk(sl, w_sb, base4, bias_base, dt, on_dve=False):
                    # transposed [d, s] projection for one head. Bias add on
                    # ACT normally; interleaved K units use DVE so they do
                    # not delay the exp stream queued on ACT.
                    xt_e = xt_tiles[sl]
                    psq = pa.tile([128, SQ], F32, name="psq", tag="pa")
                    for et in range(NET):
                        nc.tensor.matmul(
                            psq[:, :],
                            lhsT=w_sb[:, et * DL + dt * 128 : et * DL + (dt + 1) * 128],
                            rhs=xt_e[:, et * SQ : (et + 1) * SQ],
                            start=(et == 0),
                            stop=(et == NET - 1),
                        )
                    dst = qkt[:, (base4 + dt) * S + sl * SQ : (base4 + dt) * S + (sl + 1) * SQ]
                    if on_dve:
                        nc.vector.tensor_scalar_add(
                            out=dst, in0=psq[:, :],
                            scalar1=biasqk[:, bias_base + dt : bias_base + dt + 1],
                        )
                    else:
                        nc.scalar.add(
                            dst, psq[:, :],
                            biasqk[:, bias_base + dt : bias_base + dt + 1],
                        )

                def vslice(kt, h):
                    return vt[:, kt * VW + h * 129 : kt * VW + (h + 1) * 129]

                def emit_norm_chain(psO, qb, h, qc, on_act):
                    # 1/denom (col 128) times the value cols, then a PE
                    # transpose back to [d, q] for the O-projection
                    recip = rp.tile([128, 1], F32, name="recip", tag="recip")
                    nc.vector.reciprocal_approx_fast(
                        recip[:, :], psO[h, qc][:, 128:129]
                    )
                    stg = rp.tile([128, 128], BF16, name="stg", tag="stg")
                    nc.vector.tensor_scalar_mul(
                        out=stg[:, :], in0=psO[h, qc][:, 0:128],
                        scalar1=recip[:, :],
                    )
                    psT = pa.tile([128, 128], BF16, name="psT", tag="pa")
                    nc.tensor.transpose(psT[:, :], stg[:, :], idt_sb[:, :])
                    dst = outt[:, (h * NQB + qb) * QB + qc * 128 :
                               (h * NQB + qb) * QB + (qc + 1) * 128]
                    if on_act:
                        nc.scalar.copy(dst, psT[:, :])
                    else:
                        nc.vector.tensor_copy(dst, psT[:, :])

                def o_proj(qb, tail=False):
                    for j in range(2):
                        st = qb * 2 + j
                        osb = op.tile([128, D], F32, name="osb", tag="osb")
                        for et in range(4):
                            psF = pa.tile([128, 512], F32, name="psF", tag="pa")
                            for h in range(NH):
                                o_base = (h * NQB + qb) * QB + j * 128
                                nc.tensor.matmul(
                                    psF[:, :],
                                    lhsT=outt[:, o_base : o_base + 128],
                                    rhs=wot_sb[:, h * D + et * 512 : h * D + (et + 1) * 512],
                                    start=(h == 0),
                                    stop=(h == NH - 1),
                                )
                            if tail and et % 2 == 1:
                                # tail: ACT is idle — split the drain copies
                                nc.scalar.copy(
                                    osb[:, et * 512 : (et + 1) * 512], psF[:, :]
                                )
                            else:
                                nc.vector.tensor_copy(
                                    osb[:, et * 512 : (et + 1) * 512], psF[:, :]
                                )
                            if tail and et == 1:
                                nc.sync.dma_start(
                                    out=OUT[st * 128 : (st + 1) * 128, :1024],
                                    in_=osb[:, :1024],
                                )
                        if tail:
                            nc.sync.dma_start(
                                out=OUT[st * 128 : (st + 1) * 128, 1024:],
                                in_=osb[:, 1024:],
                            )
                        else:
                            nc.sync.dma_start(
                                out=OUT[st * 128 : (st + 1) * 128, :], in_=osb[:, :]
                            )

                prev_block = [None]  # (psO dict, qb) of the not-yet-drained block

                def finish_prev(tail=False):
                    # normalize+transpose and O-project the PREVIOUS block.
                    # Deferred into the NEXT block (after its first scores)
                    # so PE rolls from this block's diagonal AV straight into
                    # the next block's scores with no normalize bubble.
                    if prev_block[0] is None:
                        return
                    ppsO, pqb = prev_block[0]
                    prev_block[0] = None
                    for h in range(NH):
                        for qc in range(2):
                            emit_norm_chain(ppsO, pqb, h, qc, on_act=False)
                    o_proj(pqb, tail=tail)

                def attention_qb(qb, units=None, diag_units=None):
                    # Software-pipelined: AV matmuls run one pair behind the
                    # scores/exp stream (so PE never waits on the exp it just
                    # queued). V units of this slice land between the
                    # diagonal's scores and its AV (which needs them).
                    npairs = (qb + 1) if is_causal else NQB
                    units = list(units) if units else []
                    diag_units = list(diag_units) if diag_units else []
                    per_gap = -(-len(units) // max(1, npairs - 1)) if units else 0
                    psO = {}

                    def alloc_psO():
                        for h in range(NH):
                            for qc in range(2):
                                psO[h, qc] = pso.tile(
                                    [128, 512], F32, name="psO", tag="o"
                                )

                    def emit_scores(pi, h):
                        qb0 = h * S + qb * QB
                        psS = scor.tile([128, SQ], F32, name="psS", tag="sc")
                        p2 = pp.tile([128, SQ], BF16, name="p2", tag="p")
                        if is_causal and pi == qb:
                            # diagonal pair: tile i=0 spans the full 256
                            # (masked), tile i=1 only q-cols 128:256; one exp
                            kt0, kt1 = 2 * qb, 2 * qb + 1
                            nc.tensor.matmul(
                                psS[:, 0:QB],
                                lhsT=qkt[:, (2 + h) * S + kt0 * 128 : (2 + h) * S + (kt0 + 1) * 128],
                                rhs=qkt[:, qb0 : qb0 + QB],
                                start=True,
                                stop=True,
                            )
                            nc.tensor.matmul(
                                psS[:, QB : QB + 128],
                                lhsT=qkt[:, (2 + h) * S + kt1 * 128 : (2 + h) * S + (kt1 + 1) * 128],
                                rhs=qkt[:, qb0 + 128 : qb0 + QB],
                                start=True,
                                stop=True,
                            )
                            nc.scalar.activation(
                                p2[:, 0 : QB + 128], psS[:, 0 : QB + 128],
                                mybir.ActivationFunctionType.Exp,
                                scale=float(SCALE),
                            )
                            nc.vector.tensor_mul(
                                p2[:, 0:QB], p2[:, 0:QB], masks_sb[:, 0:QB]
                            )
                            nc.vector.tensor_mul(
                                p2[:, QB : QB + 128], p2[:, QB : QB + 128],
                                masks_sb[:, 0:128],
                            )
                        else:
                            for half in range(2):
                                kt = 2 * pi + half
                                nc.tensor.matmul(
                                    psS[:, half * QB : (half + 1) * QB],
                                    lhsT=qkt[:, (2 + h) * S + kt * 128 : (2 + h) * S + (kt + 1) * 128],
                                    rhs=qkt[:, qb0 : qb0 + QB],
                                    start=True,
                                    stop=True,
                                )
                            nc.scalar.activation(
                                p2[:, :], psS[:, :],
                                mybir.ActivationFunctionType.Exp,
                                scale=float(SCALE),
                            )
                        return p2

                    def emit_av(pi, h, p2):
                        if is_causal and pi == qb:
                            kt0, kt1 = 2 * qb, 2 * qb + 1
                            first = qb == 0
                            nc.tensor.matmul(
                                psO[h, 0][:, :129],
                                lhsT=p2[:, 0:128],
                                rhs=vslice(kt0, h),
                                start=first,
                                stop=True,
                            )
                            nc.tensor.matmul(
                                psO[h, 1][:, :129],
                                lhsT=p2[:, 128:256],
                                rhs=vslice(kt0, h),
                                start=first,
                                stop=False,
                            )
                            nc.tensor.matmul(
                                psO[h, 1][:, :129],
                                lhsT=p2[:, QB : QB + 128],
                                rhs=vslice(kt1, h),
                                start=False,
                                stop=True,
                            )
                        else:
                            last = (not is_causal) and pi == npairs - 1
                            for half in range(2):
                                kt = 2 * pi + half
                                for qc in range(2):
                                    nc.tensor.matmul(
                                        psO[h, qc][:, :129],
                                        lhsT=p2[:, half * QB + qc * 128 : half * QB + (qc + 1) * 128],
                                        rhs=vslice(kt, h),
                                        start=(pi == 0 and half == 0),
                                        stop=(last and half == 1),
                                    )

                    prev = None  # (pi, p2_h0, p2_h1)
                    for pi in range(npairs):
                        diag = is_causal and pi == qb
                        if units and diag:
                            while units:
                                units.pop(0)()
                        elif units and pi > 0:
                            for _ in range(per_gap):
                                if units:
                                    units.pop(0)()
                        p2s = [emit_scores(pi, h) for h in range(NH)]
                        if pi == 0:
                            finish_prev()
                            alloc_psO()
                        if prev is not None:
                            for h in range(NH):
                                emit_av(prev[0], h, prev[1 + h])
                            prev = None
                        if diag:
                            while diag_units:
                                diag_units.pop(0)()
                            for h in range(NH):
                                emit_av(pi, h, p2s[h])
                            prev_block[0] = (psO, qb)
                        else:
                            prev = (pi, p2s[0], p2s[1])
                    if prev is not None:  # non-causal: drain last pair
                        for h in range(NH):
                            emit_av(prev[0], h, prev[1 + h])
                    if not is_causal:
                        for h in range(NH):
                            for qc in range(2):
                                emit_norm_chain(psO, qb, h, qc, on_act=False)
                        o_proj(qb)

                if is_causal:
                    for sl in range(NSQ):
                        if sl == 0:
                            emit_v0_all()
                            for d in range(NH):
                                emit_qk0(wk_sb, 2, 2, d)
                            for d in range(NH):
                                emit_qk0(wq_sb, 0, 0, d)
                            # slice-1 X load AFTER the startup burst so its
                            # 2MB transfer does not delay the weight DMAs
                            load_xe(1)
                            attention_qb(0, [])
                            attention_qb(1, [])
                            continue
                        if sl + 1 < NSQ:
                            load_xe(sl + 1)
                        for dt in range(NH):
                            emit_qk(sl, wq_sb, 0, 0, dt)
                        units_k = [
                            (lambda s=sl, d=d: emit_qk(s, wk_sb, 2, 2, d, on_dve=True))
                            for d in range(NH)
                        ]
                        diag_a = [
                            (lambda s=sl, j=j: emit_v_tile(s, j)) for j in (0, 1)
                        ]
                        diag_b = [
                            (lambda s=sl, j=j: emit_v_tile(s, j)) for j in (2, 3)
                        ]
                        attention_qb(2 * sl, units_k, diag_a)
                        attention_qb(2 * sl + 1, [], diag_b)
                    finish_prev(tail=True)
                else:
                    for sl in range(NSQ):
                        if sl + 1 < NSQ:
                            load_xe(sl + 1)
                        if sl == 0:
                            emit_v0_all()
                            for d in range(NH):
                                emit_qk0(wk_sb, 2, 2, d)
                            for d in range(NH):
                                emit_qk0(wq_sb, 0, 0, d)
                            continue
                        for j in range(SQ // 128):
                            emit_v_tile(sl, j)
                        for w_sb, base4, bias_base in ((wq_sb, 0, 0), (wk_sb, 2, 2)):
                            for dt in range(NH):
                                emit_qk(sl, w_sb, base4, bias_base, dt)
                    for qb in range(NQB):
                        attention_qb(qb)
    nc.finalize()
    return nc


def _bf16(a: np.ndarray) -> np.ndarray:
    return np.ascontiguousarray(a.astype(ml_dtypes.bfloat16))


def make_in_maps(X, Wq, bq, Wk, bk, Wv, bv, Wo, is_causal: bool):
    x2d = np.asarray(X, dtype=np.float32).reshape(S, D)
    # xt2[sl*128+p, et*512+c] = X^T[et*128+p, sl*512+c]
    xt2 = _bf16(
        x2d.T.reshape(NET, 128, NSQ, SQ)
        .transpose(2, 1, 0, 3)
        .reshape(NSQ * 128, NET * SQ)
    )
    ki = np.arange(128)[:, None]
    qj = np.arange(QB)[None, :]
    masks = (ki <= qj).astype(ml_dtypes.bfloat16)
    idt = np.eye(128, dtype=ml_dtypes.bfloat16)

    def _pack_w(wT):  # [D, DL] -> [128, NET*DL]
        return _bf16(
            np.ascontiguousarray(wT).reshape(NET, 128, DL)
            .transpose(1, 0, 2)
            .reshape(128, NET * DL)
        )

    in_maps = []
    for c in range(NCORES):
        sl = slice(c * DL, (c + 1) * DL)
        wot = np.asarray(Wo)[:, sl].T  # [DL, D]
        wo2 = _bf16(wot.reshape(NH, 128, D).transpose(1, 0, 2).reshape(128, NH * D))
        in_maps.append(
            {
                "xt2": xt2,
                "wq2": _pack_w(np.asarray(Wq)[sl, :].T),
                "wk2": _pack_w(np.asarray(Wk)[sl, :].T),
                "wv2": _pack_w(np.asarray(Wv)[sl, :].T),
                "bqkc": np.ascontiguousarray(
                    np.stack(
                        [
                            np.asarray(bq, dtype=np.float32)[sl][:128],
                            np.asarray(bq, dtype=np.float32)[sl][128:],
                            np.asarray(bk, dtype=np.float32)[sl][:128],
                            np.asarray(bk, dtype=np.float32)[sl][128:],
                        ],
                        axis=1,
                    )
                ),
                "bvrow": _bf16(np.asarray(bv)[None, sl]),
                "wo2": wo2,
                "masks2": masks,
                "idt": idt,
            }
        )
    return in_maps


_NC_CACHE: dict = {}


def _get_nc(is_causal: bool) -> bass.Bass:
    if is_causal not in _NC_CACHE:
        _NC_CACHE[is_causal] = build_nc(is_causal)
    return _NC_CACHE[is_causal]


def kernel(X, Wq, bq, Wk, bk, Wv, bv, Wo, bo, is_causal, **run_kwargs):
    causal = bool(int(np.asarray(is_causal)))
    nc = _get_nc(causal)
    in_maps = make_in_maps(X, Wq, bq, Wk, bk, Wv, bv, Wo, causal)
    res = run_bass_kernel_spmd(nc, in_maps, core_ids=list(range(NCORES)), **run_kwargs)
    out = np.asarray(bo, dtype=np.float32)[None, :].repeat(S, axis=0)
    for c in range(NCORES):
        out += res.results[c]["out"]
    return out.reshape(1, S, D)


# revision 18
# speedup vs baseline: 1.2139x; 1.0012x over previous
"""Trainium2 Bass kernel for nn_MultiHeadAttention (B=1, S=4096, D=2048, H=16, HD=128).

Sharding: tensor-parallel over heads — 2 heads per core on 8 NeuronCores.
Each core computes its 2 heads' Q/K/V projections, causal attention, and a
partial output projection (row-split Wo); the host sums the 8 partials and
adds the output bias (the all-reduce/unshard step).

Key structural idea vs the earlier revision: the softmax denominator used to
cost a dedicated ones-column matmul per p-tile — as many PE moving cycles as
the attn@V matmul itself (~61us/core). This version computes attention in the
[q, d] orientation instead: p is the STATIONARY operand (128-q chunks) and V,
augmented with a literal ones column, is the MOVING operand. One matmul then
yields psO[q, 0:128] = p^T V and psO[q, 128] = sum_k p (the denominator) —
the denominator is free (+1 moving cycle per 128). q-blocks are 256 wide so
the four live [q, d+1] accumulators (2 heads x 2 q-chunks) plus the 2-deep
scores ring plus a 2-slot scratch ring fit the 8 PSUM banks exactly (PSUM
slots are bank-granular).

Layout/schedule (per core, matmuls bf16 with fp32 PSUM):
  - X^T streamed in eight 512-col slices (double-buffered); slice 0 and the
    V weights arrive as small leading chunks in separate tiles so the first
    V matmuls chase the DMA stream (sync-engine descriptor issue rate is the
    startup bottleneck). Projections are FUSED into attention: slice sl's Q
    is emitted first, then its V/K units interleave into attention blocks
    2sl/2sl+1 as PE fill-work while ACT catches up on the exp queue (K/V
    land before the diagonal pair that needs them). Interleaved K bias adds
    go on DVE so they do not delay the exp stream on ACT.
  - Q, K produced transposed [d, s]; scores computed transposed per k-tile
    pair into one PSUM bank, one wide exp per pair. Causal masking is
    multiplicative post-exp; the diagonal pair is narrowed triangularly
    (tile i only covers q >= 128*i) and the AV chunk matmuls narrow the
    same way for free.
  - attn@V: stationary = p [128k, 128q chunk], moving = [V_h | 1] [128k,
    129]; accumulated over all k-tiles into psO[h][qc]. Normalize =
    reciprocal of the denominator column times the 128 value columns (DVE,
    per-partition scalar), then a PE transpose (identity permutation)
    restores outt to [d, s] bf16 for the O-projection. No partition
    broadcasts, no denominator folds.
  - O-projection unchanged: out[s, e] += outt_h^T @ WoT_h accumulated over
    both heads; per s-tile the 4 PSUM results gather into one [128, 2048]
    SBUF tile, stored with a single DMA. PSUM->SBUF copies alternate
    DVE/ACT.

Build notes:
  - Bacc (not raw Bass): walrus encodes at most ONE sem wait per
    instruction; Bacc's generate_event_semaphores pass splits larger sets.
  - PSUM banks: scores ring 2 (bufs=2 x [128,512]f32) + psO 4 (bufs=4) +
    scratch ring 2 (bufs=2: proj psq/psv, O-proj psF, transpose psT) = 8.
"""

import numpy as np
import ml_dtypes

import concourse.bass as bass
import concourse.mybir as mybir
import concourse.tile as tile
from concourse import bacc
from concourse.bass_utils import run_bass_kernel_spmd


S = 4096          # sequence length
D = 2048          # model dim
NCORES = 8
DL = D // NCORES  # 256 local head dims (2 heads)
NH = 2            # heads per core
HD = 128          # head dim
QB = 256          # q block width
NQB = S // QB     # 16
KT = 128          # k tile (partitions)
NKT = S // KT     # 32
ET = 128          # e contraction tile
NET = D // ET     # 16
NST = S // 128    # 32 s-tiles
VW = 2 * (HD + 1)  # vt cols per s-tile: [h0 d0..127, 1 | h1 d0..127, 1]
SQ = 512          # X^T streaming slice width (s columns)
NSQ = S // SQ     # 8 slices
SCALE = 1.0 / np.sqrt(HD)

BF16 = mybir.dt.bfloat16
F32 = mybir.dt.float32


def build_nc(is_causal: bool) -> bass.Bass:
    nc = bacc.Bacc()

    # xt2 row-block sl: [128, et*512+c] = X[sl*512+c, et*128+p] (host packed)
    XT2 = nc.dram_tensor("xt2", [NSQ * 128, NET * SQ], BF16, kind="ExternalInput")
    # weights packed [128, et*256+c] = W^T[et*128+p, c]
    WQ2 = nc.dram_tensor("wq2", [128, NET * DL], BF16, kind="ExternalInput")
    WK2 = nc.dram_tensor("wk2", [128, NET * DL], BF16, kind="ExternalInput")
    WV2 = nc.dram_tensor("wv2", [128, NET * DL], BF16, kind="ExternalInput")
    # bias columns [128, 4]: bq.d0 | bq.d1 | bk.d0 | bk.d1
    BQKC = nc.dram_tensor("bqkc", [128, 4], F32, kind="ExternalInput")
    BVROW = nc.dram_tensor("bvrow", [1, DL], BF16, kind="ExternalInput")
    # [128, h*2048+c] = Wo^T[h*128+p, c]
    WO2 = nc.dram_tensor("wo2", [128, NH * D], BF16, kind="ExternalInput")
    # [128, 256] multiplicative causal mask: m[k, c] = (k <= c)
    MASKS2 = nc.dram_tensor("masks2", [128, QB], BF16, kind="ExternalInput")
    IDT = nc.dram_tensor("idt", [128, 128], BF16, kind="ExternalInput")
    # bf16 partials: host sums 8 of them in f32; the ~0.4% partial
    # quantization is well inside the error budget and halves store traffic
    OUT = nc.dram_tensor("out", [S, D], BF16, kind="ExternalOutput")

    with tile.TileContext(nc) as tc:
        with tc.tile_pool(name="persist", bufs=1) as persist:
            # Q head0 | Q head1 | K head0 | K head1, each [128, 4096]
            qkt = persist.tile([128, 4 * S], BF16, name="qkt")
            # V with interleaved ones cols: s-tile st at [st*VW, (st+1)*VW),
            # head h at +h*129; col +h*129+128 stays 1.0 (memset below)
            vt = persist.tile([128, NST * VW], BF16, name="vt")
            wot_sb = persist.tile([128, NH * D], BF16, name="wot_sb")
            masks_sb = persist.tile([128, QB], BF16, name="masks_sb")
            wk_sb = persist.tile([128, NET * DL], BF16, name="wk_sb")
            wq_sb = persist.tile([128, NET * DL], BF16, name="wq_sb")
            idt_sb = persist.tile([128, 128], BF16, name="idt_sb")
            biasqk = persist.tile([128, 4], F32, name="biasqk")
            bvrow_sb = persist.tile([1, DL], BF16, name="bvrow_sb")
            bvb_sb = persist.tile([128, DL], BF16, name="bvb_sb")
            # normalized attention outputs, transposed: (h*NQB+qb) tile [128d, 256q]
            outt = persist.tile([128, NH * NQB * QB], BF16, name="outt")

            # pre-set the interleaved ones columns (V writes overwrite d-cols)
            nc.vector.memset(vt[:, :], 1.0)

            with tc.tile_pool(name="xtp", bufs=2) as xtp, \
                 tc.tile_pool(name="scor", bufs=2, space="PSUM") as scor, \
                 tc.tile_pool(name="pso", bufs=4, space="PSUM") as pso, \
                 tc.tile_pool(name="pa", bufs=2, space="PSUM") as pa, \
                 tc.tile_pool(name="pp", bufs=8) as pp, \
                 tc.tile_pool(name="rp", bufs=4) as rp, \
                 tc.tile_pool(name="op", bufs=2) as op:

                # DMA order: V weights + X^T slice 0, chunked and interleaved
                # so the first V matmuls start after ~1 MB; then the rest.
                xt_tiles = {}
                def load_xe(sl):
                    xt_e = xtp.tile([128, NET * SQ], BF16, name="xt_e", tag="xt")
                    xt_tiles[sl] = xt_e
                    nc.sync.dma_start(
                        out=xt_e[:, :], in_=XT2[sl * 128 : (sl + 1) * 128, :]
                    )
                XCHUNKS = [2, 2, 4, 4, 4]           # et tiles per chunk
                XOFF = [0, 2, 4, 8, 12]             # et offset per chunk
                def chunk_of(et):
                    for ci in range(len(XCHUNKS) - 1, -1, -1):
                        if et >= XOFF[ci]:
                            return ci, et - XOFF[ci]
                xt0c = [
                    persist.tile([128, n * SQ], BF16, name=f"xt0c{c}")
                    for c, n in enumerate(XCHUNKS)
                ]
                wv_cs = [
                    persist.tile([128, n * DL], BF16, name=f"wv_c{c}")
                    for c, n in enumerate(XCHUNKS)
                ]
                # Startup burst on TWO descriptor queues (Sync + ACT, both
                # HWDGE): X chunks + wk on Sync, wv chunks + wq + small
                # tensors on ACT.  Doubles the ~0.65us/descriptor issue rate
                # and runs two DMA rings in parallel — the slice-0 V chase
                # is otherwise transfer-bound.  wk/wq in halves (subtile
                # deps let the first 8 et-matmuls start early).
                HNW = NET * DL // 2
                for ci, n in enumerate(XCHUNKS):
                    nc.sync.dma_start(
                        out=xt0c[ci][:, :],
                        in_=XT2[0:128, XOFF[ci] * SQ : (XOFF[ci] + n) * SQ],
                    )
                    nc.scalar.dma_start(
                        out=wv_cs[ci][:, :],
                        in_=WV2[:, XOFF[ci] * DL : (XOFF[ci] + n) * DL],
                    )
                nc.sync.dma_start(out=wk_sb[:, :HNW], in_=WK2[:, :HNW])
                nc.sync.dma_start(out=wk_sb[:, HNW:], in_=WK2[:, HNW:])
                nc.scalar.dma_start(out=wq_sb[:, :HNW], in_=WQ2[:, :HNW])
                nc.scalar.dma_start(out=wq_sb[:, HNW:], in_=WQ2[:, HNW:])
                nc.scalar.dma_start(out=biasqk[:, :], in_=BQKC[:, :])
                if is_causal:
                    nc.scalar.dma_start(out=masks_sb[:, :], in_=MASKS2[:, :])
                nc.scalar.dma_start(out=bvrow_sb[:, :], in_=BVROW[:, :])
                nc.scalar.dma_start(out=idt_sb[:, :], in_=IDT[:, :])
                nc.scalar.dma_start(out=wot_sb[:, :], in_=WO2[:, :])
                # broadcast bv across partitions once; folded into each V
                # tile's PSUM->SBUF copy below
                nc.gpsimd.partition_broadcast(bvb_sb[:, :], bvrow_sb[:, :])

                def store_v(psv, st):
                    # psv [128, 256] f32 -> vt d-cols (ones cols untouched)
                    for h in range(NH):
                        nc.vector.scalar_tensor_tensor(
                            out=vt[:, st * VW + h * 129 : st * VW + h * 129 + 128],
                            in0=psv[:, h * 128 : (h + 1) * 128],
                            scalar=1.0,
                            in1=bvb_sb[:, h * 128 : (h + 1) * 128],
                            op0=mybir.AluOpType.mult,
                            op1=mybir.AluOpType.add,
                        )

                def emit_v0_all():
                    # et-major over all 4 s-tiles so each DMA chunk is
                    # consumed as late as possible (pso ring is free here)
                    psvs = [
                        pso.tile([128, 512], F32, name="psv0", tag="o")
                        for _ in range(SQ // 128)
                    ]
                    for et in range(NET):
                        ci, le = chunk_of(et)
                        for stl in range(SQ // 128):
                            nc.tensor.matmul(
                                psvs[stl][:, :DL],
                                lhsT=xt0c[ci][:, le * SQ + stl * 128 : le * SQ + (stl + 1) * 128],
                                rhs=wv_cs[ci][:, le * DL : (le + 1) * DL],
                                start=(et == 0),
                                stop=(et == NET - 1),
                            )
                    for stl in range(SQ // 128):
                        store_v(psvs[stl][:, :DL], stl)

                def emit_qk0(w_sb, base4, bias_base, dt):
                    psq = scor.tile([128, SQ], F32, name="psq0", tag="sc")
                    for et in range(NET):
                        ci, le = chunk_of(et)
                        nc.tensor.matmul(
                            psq[:, :SQ],
                            lhsT=w_sb[:, et * DL + dt * 128 : et * DL + (dt + 1) * 128],
                            rhs=xt0c[ci][:, le * SQ : (le + 1) * SQ],
                            start=(et == 0),
                            stop=(et == NET - 1),
                        )
                    nc.scalar.add(
                        qkt[:, (base4 + dt) * S : (base4 + dt) * S + SQ],
                        psq[:, :SQ],
                        biasqk[:, bias_base + dt : bias_base + dt + 1],
                    )

                def emit_v_tile(sl, stl):
                    xt_e = xt_tiles[sl]
                    st = sl * (SQ // 128) + stl
                    psv = pa.tile([128, DL], F32, name="psv", tag="pa")
                    for et in range(NET):
                        ci, le = chunk_of(et)
                        nc.tensor.matmul(
                            psv[:, :DL],
                            lhsT=xt_e[:, et * SQ + stl * 128 : et * SQ + (stl + 1) * 128],
                            rhs=wv_cs[ci][:, le * DL : (le + 1) * DL],
                            start=(et == 0),
                            stop=(et == NET - 1),
                        )
                    store_v(psv[:, :DL], st)

                def emit_qk(sl, w_sb, base4, bias_base, dt, on_dve=False):
                    # transposed [d, s] projection for one head. Bias add on
                    # ACT normally; interleaved K units use DVE so they do
                    # not delay the exp stream queued on ACT.
                    xt_e = xt_tiles[sl]
                    psq = pa.tile([128, SQ], F32, name="psq", tag="pa")
                    for et in range(NET):
                        nc.tensor.matmul(
                            psq[:, :],
                            lhsT=w_sb[:, et * DL + dt * 128 : et * DL + (dt + 1) * 128],
                            rhs=xt_e[:, et * SQ : (et + 1) * SQ],
                            start=(et == 0),
                            stop=(et == NET - 1),
                        )
                    dst = qkt[:, (base4 + dt) * S + sl * SQ : (base4 + dt) * S + (sl + 1) * SQ]
                    if on_dve:
                        nc.vector.tensor_scalar_add(
                            out=dst, in0=psq[:, :],
                            scalar1=biasqk[:, bias_base + dt : bias_base + dt + 1],
                        )
                    else:
                        nc.scalar.add(
                            dst, psq[:, :],
                            biasqk[:, bias_base + dt : bias_base + dt + 1],
                        )

                def vslice(kt, h):
                    return vt[:, kt * VW + h * 129 : kt * VW + (h + 1) * 129]

                def emit_norm_chain(psO, qb, h, qc, on_act):
                    # 1/denom (col 128) times the value cols, then a PE
                    # transpose back to [d, q] for the O-projection
                    recip = rp.tile([128, 1], F32, name="recip", tag="recip")
                    nc.vector.reciprocal_approx_fast(
                        recip[:, :], psO[h, qc][:, 128:129]
                    )
                    stg = rp.tile([128, 128], BF16, name="stg", tag="stg")
                    nc.vector.tensor_scalar_mul(
                        out=stg[:, :], in0=psO[h, qc][:, 0:128],
                        scalar1=recip[:, :],
                    )
                    psT = pa.tile([128, 128], BF16, name="psT", tag="pa")
                    nc.tensor.transpose(psT[:, :], stg[:, :], idt_sb[:, :])
                    dst = outt[:, (h * NQB + qb) * QB + qc * 128 :
                               (h * NQB + qb) * QB + (qc + 1) * 128]
                    if on_act:
                        nc.scalar.copy(dst, psT[:, :])
                    else:
                        nc.vector.tensor_copy(dst, psT[:, :])

                def o_proj(qb, tail=False):
                    for j in range(2):
                        st = qb * 2 + j
                        osb = op.tile([128, D], BF16, name="osb", tag="osb")
                        for et in range(4):
                            psF = pa.tile([128, 512], F32, name="psF", tag="pa")
                            for h in range(NH):
                                o_base = (h * NQB + qb) * QB + j * 128
                                nc.tensor.matmul(
                                    psF[:, :],
                                    lhsT=outt[:, o_base : o_base + 128],
                                    rhs=wot_sb[:, h * D + et * 512 : h * D + (et + 1) * 512],
                                    start=(h == 0),
                                    stop=(h == NH - 1),
                                )
                            if tail and et % 2 == 1:
                                # tail: ACT is idle — split the drain copies
                                nc.scalar.copy(
                                    osb[:, et * 512 : (et + 1) * 512], psF[:, :]
                                )
                            else:
                                nc.vector.tensor_copy(
                                    osb[:, et * 512 : (et + 1) * 512], psF[:, :]
                                )
                            if tail and et == 1:
                                nc.sync.dma_start(
                                    out=OUT[st * 128 : (st + 1) * 128, :1024],
                                    in_=osb[:, :1024],
                                )
                        if tail:
                            nc.sync.dma_start(
                                out=OUT[st * 128 : (st + 1) * 128, 1024:],
                                in_=osb[:, 1024:],
                            )
                        else:
                            nc.sync.dma_start(
                                out=OUT[st * 128 : (st + 1) * 128, :], in_=osb[:, :]
                            )

                prev_block = [None]  # (psO dict, qb) of the not-yet-drained block

                def finish_prev(tail=False):
                    # normalize+transpose and O-project the PREVIOUS block.
                    # Deferred into the NEXT block (after its first scores)
                    # so PE rolls from this block's diagonal AV straight into
                    # the next block's scores with no normalize bubble.
                    if prev_block[0] is None:
                        return
                    ppsO, pqb = prev_block[0]
                    prev_block[0] = None
                    for h in range(NH):
                        for qc in range(2):
                            emit_norm_chain(ppsO, pqb, h, qc, on_act=False)
                    o_proj(pqb, tail=tail)

                def attention_qb(qb, units=None, diag_units=None):
                    # Software-pipelined: AV matmuls run one pair behind the
                    # scores/exp stream (so PE never waits on the exp it just
                    # queued). V units of this slice land between the
                    # diagonal's scores and its AV (which needs them).
                    npairs = (qb + 1) if is_causal else NQB
                    units = list(units) if units else []
                    diag_units = list(diag_units) if diag_units else []
                    per_gap = -(-len(units) // max(1, npairs - 1)) if units else 0
                    psO = {}

                    def alloc_psO():
                        for h in range(NH):
                            for qc in range(2):
                                psO[h, qc] = pso.tile(
                                    [128, 512], F32, name="psO", tag="o"
                                )

                    def emit_scores(pi, h):
                        qb0 = h * S + qb * QB
                        psS = scor.tile([128, SQ], F32, name="psS", tag="sc")
                        p2 = pp.tile([128, SQ], BF16, name="p2", tag="p")
                        if is_causal and pi == qb:
                            # diagonal pair: tile i=0 spans the full 256
                            # (masked), tile i=1 only q-cols 128:256; one exp
                            kt0, kt1 = 2 * qb, 2 * qb + 1
                            nc.tensor.matmul(
                                psS[:, 0:QB],
                                lhsT=qkt[:, (2 + h) * S + kt0 * 128 : (2 + h) * S + (kt0 + 1) * 128],
                                rhs=qkt[:, qb0 : qb0 + QB],
                                start=True,
                                stop=True,
                            )
                            nc.tensor.matmul(
                                psS[:, QB : QB + 128],
                                lhsT=qkt[:, (2 + h) * S + kt1 * 128 : (2 + h) * S + (kt1 + 1) * 128],
                                rhs=qkt[:, qb0 + 128 : qb0 + QB],
                                start=True,
                                stop=True,
                            )
                            nc.scalar.activation(
                                p2[:, 0 : QB + 128], psS[:, 0 : QB + 128],
                                mybir.ActivationFunctionType.Exp,
                                scale=float(SCALE),
                            )
                            nc.vector.tensor_mul(
                                p2[:, 0:QB], p2[:, 0:QB], masks_sb[:, 0:QB]
                            )
                            nc.vector.tensor_mul(
                                p2[:, QB : QB + 128], p2[:, QB : QB + 128],
                                masks_sb[:, 0:128],
                            )
                        else:
                            for half in range(2):
                                kt = 2 * pi + half
                                nc.tensor.matmul(
                                    psS[:, half * QB : (half + 1) * QB],
                                    lhsT=qkt[:, (2 + h) * S + kt * 128 : (2 + h) * S + (kt + 1) * 128],
                                    rhs=qkt[:, qb0 : qb0 + QB],
                                    start=True,
                                    stop=True,
                                )
                            nc.scalar.activation(
                                p2[:, :], psS[:, :],
                                mybir.ActivationFunctionType.Exp,
                                scale=float(SCALE),
                            )
                        return p2

                    def emit_av(pi, h, p2):
                        if is_causal and pi == qb:
                            kt0, kt1 = 2 * qb, 2 * qb + 1
                            first = qb == 0
                            nc.tensor.matmul(
                                psO[h, 0][:, :129],
                                lhsT=p2[:, 0:128],
                                rhs=vslice(kt0, h),
                                start=first,
                                stop=True,
                            )
                            nc.tensor.matmul(
                                psO[h, 1][:, :129],
                                lhsT=p2[:, 128:256],
                                rhs=vslice(kt0, h),
                                start=first,
                                stop=False,
                            )
                            nc.tensor.matmul(
                                psO[h, 1][:, :129],
                                lhsT=p2[:, QB : QB + 128],
                                rhs=vslice(kt1, h),
                                start=False,
                                stop=True,
                            )
                        else:
                            last = (not is_causal) and pi == npairs - 1
                            for half in range(2):
                                kt = 2 * pi + half
                                for qc in range(2):
                                    nc.tensor.matmul(
                                        psO[h, qc][:, :129],
                                        lhsT=p2[:, half * QB + qc * 128 : half * QB + (qc + 1) * 128],
                                        rhs=vslice(kt, h),
                                        start=(pi == 0 and half == 0),
                                        stop=(last and half == 1),
                                    )

                    prev = None  # (pi, p2_h0, p2_h1)
                    for pi in range(npairs):
                        diag = is_causal and pi == qb
                        if units and diag:
                            while units:
                                units.pop(0)()
                        elif units and pi > 0:
                            for _ in range(per_gap):
                                if units:
                                    units.pop(0)()
                        p2s = [emit_scores(pi, h) for h in range(NH)]
                        if pi == 0:
                            finish_prev()
                            alloc_psO()
                        if prev is not None:
                            for h in range(NH):
                                emit_av(prev[0], h, prev[1 + h])
                            prev = None
                        if diag:
                            while diag_units:
                                diag_units.pop(0)()
                            for h in range(NH):
                                emit_av(pi, h, p2s[h])
                            prev_block[0] = (psO, qb)
                        else:
                            prev = (pi, p2s[0], p2s[1])
                    if prev is not None:  # non-causal: drain last pair
                        for h in range(NH):
                            emit_av(prev[0], h, prev[1 + h])
                    if not is_causal:
                        for h in range(NH):
                            for qc in range(2):
                                emit_norm_chain(psO, qb, h, qc, on_act=False)
                        o_proj(qb)

                if is_causal:
                    # Q of slice sl+1 is emitted as gap-fill units inside
                    # block 2sl+1 (always before block 2sl+2 needs it)
                    def q_units(sl):
                        if sl >= NSQ:
                            return []
                        return [
                            (lambda s=sl, d=d: emit_qk(s, wq_sb, 0, 0, d, on_dve=True))
                            for d in range(NH)
                        ]
                    for sl in range(NSQ):
                        if sl == 0:
                            emit_v0_all()
                            for d in range(NH):
                                emit_qk0(wk_sb, 2, 2, d)
                            for d in range(NH):
                                emit_qk0(wq_sb, 0, 0, d)
                            # slice-1 X load AFTER the startup burst so its
                            # 2MB transfer does not delay the weight DMAs
                            load_xe(1)
                            attention_qb(0, [])
                            attention_qb(1, q_units(1))
                            continue
                        if sl + 1 < NSQ:
                            load_xe(sl + 1)
                        units_k = [
                            (lambda s=sl, d=d: emit_qk(s, wk_sb, 2, 2, d, on_dve=True))
                            for d in range(NH)
                        ]
                        diag_a = [
                            (lambda s=sl, j=j: emit_v_tile(s, j)) for j in (0, 1)
                        ]
                        diag_b = [
                            (lambda s=sl, j=j: emit_v_tile(s, j)) for j in (2, 3)
                        ]
                        attention_qb(2 * sl, units_k, diag_a)
                        attention_qb(2 * sl + 1, q_units(sl + 1), diag_b)
                    finish_prev(tail=True)
                else:
                    for sl in range(NSQ):
                        if sl + 1 < NSQ:
                            load_xe(sl + 1)
                        if sl == 0:
                            emit_v0_all()
                            for d in range(NH):
                                emit_qk0(wk_sb, 2, 2, d)
                            for d in range(NH):
                                emit_qk0(wq_sb, 0, 0, d)
                            continue
                        for j in range(SQ // 128):
                            emit_v_tile(sl, j)
                        for w_sb, base4, bias_base in ((wq_sb, 0, 0), (wk_sb, 2, 2)):
                            for dt in range(NH):
                                emit_qk(sl, w_sb, base4, bias_base, dt)
                    for qb in range(NQB):
                        attention_qb(qb)
    nc.finalize()
    return nc


def _bf16(a: np.ndarray) -> np.ndarray:
    return np.ascontiguousarray(a.astype(ml_dtypes.bfloat16))


def make_in_maps(X, Wq, bq, Wk, bk, Wv, bv, Wo, is_causal: bool):
    x2d = np.asarray(X, dtype=np.float32).reshape(S, D)
    # xt2[sl*128+p, et*512+c] = X^T[et*128+p, sl*512+c]
    xt2 = _bf16(
        x2d.T.reshape(NET, 128, NSQ, SQ)
        .transpose(2, 1, 0, 3)
        .reshape(NSQ * 128, NET * SQ)
    )
    ki = np.arange(128)[:, None]
    qj = np.arange(QB)[None, :]
    masks = (ki <= qj).astype(ml_dtypes.bfloat16)
    idt = np.eye(128, dtype=ml_dtypes.bfloat16)

    def _pack_w(wT):  # [D, DL] -> [128, NET*DL]
        return _bf16(
            np.ascontiguousarray(wT).reshape(NET, 128, DL)
            .transpose(1, 0, 2)
            .reshape(128, NET * DL)
        )

    in_maps = []
    for c in range(NCORES):
        sl = slice(c * DL, (c + 1) * DL)
        wot = np.asarray(Wo)[:, sl].T  # [DL, D]
        wo2 = _bf16(wot.reshape(NH, 128, D).transpose(1, 0, 2).reshape(128, NH * D))
        in_maps.append(
            {
                "xt2": xt2,
                "wq2": _pack_w(np.asarray(Wq)[sl, :].T),
                "wk2": _pack_w(np.asarray(Wk)[sl, :].T),
                "wv2": _pack_w(np.asarray(Wv)[sl, :].T),
                "bqkc": np.ascontiguousarray(
                    np.stack(
                        [
                            np.asarray(bq, dtype=np.float32)[sl][:128],
                            np.asarray(bq, dtype=np.float32)[sl][128:],
                            np.asarray(bk, dtype=np.float32)[sl][:128],
                            np.asarray(bk, dtype=np.float32)[sl][128:],
                        ],
                        axis=1,
                    )
                ),
                "bvrow": _bf16(np.asarray(bv)[None, sl]),
                "wo2": wo2,
                "masks2": masks,
                "idt": idt,
            }
        )
    return in_maps


_NC_CACHE: dict = {}


def _get_nc(is_causal: bool) -> bass.Bass:
    if is_causal not in _NC_CACHE:
        _NC_CACHE[is_causal] = build_nc(is_causal)
    return _NC_CACHE[is_causal]


def kernel(X, Wq, bq, Wk, bk, Wv, bv, Wo, bo, is_causal, **run_kwargs):
    causal = bool(int(np.asarray(is_causal)))
    nc = _get_nc(causal)
    in_maps = make_in_maps(X, Wq, bq, Wk, bk, Wv, bv, Wo, causal)
    res = run_bass_kernel_spmd(nc, in_maps, core_ids=list(range(NCORES)), **run_kwargs)
    out = np.asarray(bo, dtype=np.float32)[None, :].repeat(S, axis=0)
    for c in range(NCORES):
        out += np.asarray(res.results[c]["out"], dtype=np.float32)
    return out.reshape(1, S, D)


# revision 20
# speedup vs baseline: 1.2235x; 1.0079x over previous
"""Trainium2 Bass kernel for nn_MultiHeadAttention (B=1, S=4096, D=2048, H=16, HD=128).

Sharding: tensor-parallel over heads — 2 heads per core on 8 NeuronCores.
Each core computes its 2 heads' Q/K/V projections, causal attention, and a
partial output projection (row-split Wo); the host sums the 8 partials and
adds the output bias (the all-reduce/unshard step).

Key structural idea vs the earlier revision: the softmax denominator used to
cost a dedicated ones-column matmul per p-tile — as many PE moving cycles as
the attn@V matmul itself (~61us/core). This version computes attention in the
[q, d] orientation instead: p is the STATIONARY operand (128-q chunks) and V,
augmented with a literal ones column, is the MOVING operand. One matmul then
yields psO[q, 0:128] = p^T V and psO[q, 128] = sum_k p (the denominator) —
the denominator is free (+1 moving cycle per 128). q-blocks are 256 wide so
the four live [q, d+1] accumulators (2 heads x 2 q-chunks) plus the 2-deep
scores ring plus a 2-slot scratch ring fit the 8 PSUM banks exactly (PSUM
slots are bank-granular).

Layout/schedule (per core, matmuls bf16 with fp32 PSUM):
  - X^T streamed in eight 512-col slices (double-buffered); slice 0 and the
    V weights arrive as small leading chunks in separate tiles so the first
    V matmuls chase the DMA stream (sync-engine descriptor issue rate is the
    startup bottleneck). Projections are FUSED into attention: slice sl's Q
    is emitted first, then its V/K units interleave into attention blocks
    2sl/2sl+1 as PE fill-work while ACT catches up on the exp queue (K/V
    land before the diagonal pair that needs them). Interleaved K bias adds
    go on DVE so they do not delay the exp stream on ACT.
  - Q, K produced transposed [d, s]; scores computed transposed per k-tile
    pair into one PSUM bank, one wide exp per pair. Causal masking is
    multiplicative post-exp; the diagonal pair is narrowed triangularly
    (tile i only covers q >= 128*i) and the AV chunk matmuls narrow the
    same way for free.
  - attn@V: stationary = p [128k, 128q chunk], moving = [V_h | 1] [128k,
    129]; accumulated over all k-tiles into psO[h][qc]. Normalize =
    reciprocal of the denominator column times the 128 value columns (DVE,
    per-partition scalar), then a PE transpose (identity permutation)
    restores outt to [d, s] bf16 for the O-projection. No partition
    broadcasts, no denominator folds.
  - O-projection unchanged: out[s, e] += outt_h^T @ WoT_h accumulated over
    both heads; per s-tile the 4 PSUM results gather into one [128, 2048]
    SBUF tile, stored with a single DMA. PSUM->SBUF copies alternate
    DVE/ACT.

Build notes:
  - Bacc (not raw Bass): walrus encodes at most ONE sem wait per
    instruction; Bacc's generate_event_semaphores pass splits larger sets.
  - PSUM banks: scores ring 2 (bufs=2 x [128,512]f32) + psO 4 (bufs=4) +
    scratch ring 2 (bufs=2: proj psq/psv, O-proj psF, transpose psT) = 8.
"""

import numpy as np
import ml_dtypes

import concourse.bass as bass
import concourse.mybir as mybir
import concourse.tile as tile
from concourse import bacc
from concourse.bass_utils import run_bass_kernel_spmd


S = 4096          # sequence length
D = 2048          # model dim
NCORES = 8
DL = D // NCORES  # 256 local head dims (2 heads)
NH = 2            # heads per core
HD = 128          # head dim
QB = 256          # q block width
NQB = S // QB     # 16
KT = 128          # k tile (partitions)
NKT = S // KT     # 32
ET = 128          # e contraction tile
NET = D // ET     # 16
NST = S // 128    # 32 s-tiles
VW = 2 * (HD + 1)  # vt cols per s-tile: [h0 d0..127, 1 | h1 d0..127, 1]
SQ = 512          # X^T streaming slice width (s columns)
NSQ = S // SQ     # 8 slices
SCALE = 1.0 / np.sqrt(HD)

BF16 = mybir.dt.bfloat16
F32 = mybir.dt.float32


def build_nc(is_causal: bool) -> bass.Bass:
    nc = bacc.Bacc()

    # xt2 row-block sl: [128, et*512+c] = X[sl*512+c, et*128+p] (host packed)
    XT2 = nc.dram_tensor("xt2", [NSQ * 128, NET * SQ], BF16, kind="ExternalInput")
    # weights packed [128, et*256+c] = W^T[et*128+p, c]
    WQ2 = nc.dram_tensor("wq2", [128, NET * DL], BF16, kind="ExternalInput")
    WK2 = nc.dram_tensor("wk2", [128, NET * DL], BF16, kind="ExternalInput")
    WV2 = nc.dram_tensor("wv2", [128, NET * DL], BF16, kind="ExternalInput")
    # bias columns [128, 4]: bq.d0 | bq.d1 | bk.d0 | bk.d1
    BQKC = nc.dram_tensor("bqkc", [128, 4], F32, kind="ExternalInput")
    BVROW = nc.dram_tensor("bvrow", [1, DL], BF16, kind="ExternalInput")
    # [128, h*2048+c] = Wo^T[h*128+p, c]
    WO2 = nc.dram_tensor("wo2", [128, NH * D], BF16, kind="ExternalInput")
    # [128, 256] multiplicative causal mask: m[k, c] = (k <= c)
    MASKS2 = nc.dram_tensor("masks2", [128, QB], BF16, kind="ExternalInput")
    IDT = nc.dram_tensor("idt", [128, 128], BF16, kind="ExternalInput")
    # bf16 partials: host sums 8 of them in f32; the ~0.4% partial
    # quantization is well inside the error budget and halves store traffic
    OUT = nc.dram_tensor("out", [S, D], BF16, kind="ExternalOutput")

    with tile.TileContext(nc) as tc:
        with tc.tile_pool(name="persist", bufs=1) as persist:
            # Q head0 | Q head1 | K head0 | K head1, each [128, 4096]
            qkt = persist.tile([128, 4 * S], BF16, name="qkt")
            # V with interleaved ones cols: s-tile st at [st*VW, (st+1)*VW),
            # head h at +h*129; col +h*129+128 stays 1.0 (memset below)
            vt = persist.tile([128, NST * VW], BF16, name="vt")
            wot_sb = persist.tile([128, NH * D], BF16, name="wot_sb")
            masks_sb = persist.tile([128, QB], BF16, name="masks_sb")
            wk_sb = persist.tile([128, NET * DL], BF16, name="wk_sb")
            wq_sb = persist.tile([128, NET * DL], BF16, name="wq_sb")
            idt_sb = persist.tile([128, 128], BF16, name="idt_sb")
            biasqk = persist.tile([128, 4], F32, name="biasqk")
            bvrow_sb = persist.tile([1, DL], BF16, name="bvrow_sb")
            bvb_sb = persist.tile([128, DL], BF16, name="bvb_sb")
            # normalized attention outputs, transposed: (h*NQB+qb) tile [128d, 256q]
            outt = persist.tile([128, NH * NQB * QB], BF16, name="outt")

            # pre-set the interleaved ones columns (V writes overwrite the
            # d-cols); split so the first store_v only waits ~1/8 of it
            for mc in range(8):
                nc.vector.memset(
                    vt[:, mc * (NST // 8) * VW : (mc + 1) * (NST // 8) * VW], 1.0
                )

            with tc.tile_pool(name="xtp", bufs=2) as xtp, \
                 tc.tile_pool(name="scor", bufs=2, space="PSUM") as scor, \
                 tc.tile_pool(name="pso", bufs=4, space="PSUM") as pso, \
                 tc.tile_pool(name="pa", bufs=2, space="PSUM") as pa, \
                 tc.tile_pool(name="pp", bufs=8) as pp, \
                 tc.tile_pool(name="rp", bufs=4) as rp, \
                 tc.tile_pool(name="op", bufs=2) as op:

                # DMA order: V weights + X^T slice 0, chunked and interleaved
                # so the first V matmuls start after ~1 MB; then the rest.
                xt_tiles = {}
                def load_xe(sl):
                    xt_e = xtp.tile([128, NET * SQ], BF16, name="xt_e", tag="xt")
                    xt_tiles[sl] = xt_e
                    nc.sync.dma_start(
                        out=xt_e[:, :], in_=XT2[sl * 128 : (sl + 1) * 128, :]
                    )
                XCHUNKS = [2, 2, 4, 4, 4]           # et tiles per chunk
                XOFF = [0, 2, 4, 8, 12]             # et offset per chunk
                def chunk_of(et):
                    for ci in range(len(XCHUNKS) - 1, -1, -1):
                        if et >= XOFF[ci]:
                            return ci, et - XOFF[ci]
                xt0c = [
                    persist.tile([128, n * SQ], BF16, name=f"xt0c{c}")
                    for c, n in enumerate(XCHUNKS)
                ]
                wv_cs = [
                    persist.tile([128, n * DL], BF16, name=f"wv_c{c}")
                    for c, n in enumerate(XCHUNKS)
                ]
                # wk/wq in halves (subtile deps let the first 8 et-matmuls
                # start early), interleaved into the chunk stream by the
                # PE-time each transfer is needed
                HNW = NET * DL // 2
                def chunk_dma(ci):
                    n = XCHUNKS[ci]
                    nc.sync.dma_start(
                        out=wv_cs[ci][:, :],
                        in_=WV2[:, XOFF[ci] * DL : (XOFF[ci] + n) * DL],
                    )
                    nc.sync.dma_start(
                        out=xt0c[ci][:, :],
                        in_=XT2[0:128, XOFF[ci] * SQ : (XOFF[ci] + n) * SQ],
                    )
                chunk_dma(0)
                chunk_dma(1)
                chunk_dma(2)
                nc.sync.dma_start(out=wk_sb[:, :HNW], in_=WK2[:, :HNW])
                chunk_dma(3)
                nc.sync.dma_start(out=wk_sb[:, HNW:], in_=WK2[:, HNW:])
                nc.sync.dma_start(out=wq_sb[:, :HNW], in_=WQ2[:, :HNW])
                chunk_dma(4)
                nc.sync.dma_start(out=wq_sb[:, HNW:], in_=WQ2[:, HNW:])
                nc.sync.dma_start(out=biasqk[:, :], in_=BQKC[:, :])
                if is_causal:
                    nc.sync.dma_start(out=masks_sb[:, :], in_=MASKS2[:, :])
                nc.sync.dma_start(out=bvrow_sb[:, :], in_=BVROW[:, :])
                nc.sync.dma_start(out=idt_sb[:, :], in_=IDT[:, :])
                nc.sync.dma_start(out=wot_sb[:, :], in_=WO2[:, :])
                # broadcast bv across partitions once; folded into each V
                # tile's PSUM->SBUF copy below
                nc.gpsimd.partition_broadcast(bvb_sb[:, :], bvrow_sb[:, :])

                def store_v(psv, st):
                    # psv [128, 256] f32 -> vt d-cols (ones cols untouched)
                    for h in range(NH):
                        nc.vector.scalar_tensor_tensor(
                            out=vt[:, st * VW + h * 129 : st * VW + h * 129 + 128],
                            in0=psv[:, h * 128 : (h + 1) * 128],
                            scalar=1.0,
                            in1=bvb_sb[:, h * 128 : (h + 1) * 128],
                            op0=mybir.AluOpType.mult,
                            op1=mybir.AluOpType.add,
                        )

                def emit_v0_all():
                    # et-major over all 4 s-tiles so each DMA chunk is
                    # consumed as late as possible (pso ring is free here)
                    psvs = [
                        pso.tile([128, 512], F32, name="psv0", tag="o")
                        for _ in range(SQ // 128)
                    ]
                    for et in range(NET):
                        ci, le = chunk_of(et)
                        for stl in range(SQ // 128):
                            nc.tensor.matmul(
                                psvs[stl][:, :DL],
                                lhsT=xt0c[ci][:, le * SQ + stl * 128 : le * SQ + (stl + 1) * 128],
                                rhs=wv_cs[ci][:, le * DL : (le + 1) * DL],
                                start=(et == 0),
                                stop=(et == NET - 1),
                            )
                    for stl in range(SQ // 128):
                        store_v(psvs[stl][:, :DL], stl)

                def emit_qk0(w_sb, base4, bias_base, dt):
                    psq = scor.tile([128, SQ], F32, name="psq0", tag="sc")
                    for et in range(NET):
                        ci, le = chunk_of(et)
                        nc.tensor.matmul(
                            psq[:, :SQ],
                            lhsT=w_sb[:, et * DL + dt * 128 : et * DL + (dt + 1) * 128],
                            rhs=xt0c[ci][:, le * SQ : (le + 1) * SQ],
                            start=(et == 0),
                            stop=(et == NET - 1),
                        )
                    nc.scalar.add(
                        qkt[:, (base4 + dt) * S : (base4 + dt) * S + SQ],
                        psq[:, :SQ],
                        biasqk[:, bias_base + dt : bias_base + dt + 1],
                    )

                def emit_v_tile(sl, stl):
                    xt_e = xt_tiles[sl]
                    st = sl * (SQ // 128) + stl
                    psv = pa.tile([128, DL], F32, name="psv", tag="pa")
                    for et in range(NET):
                        ci, le = chunk_of(et)
                        nc.tensor.matmul(
                            psv[:, :DL],
                            lhsT=xt_e[:, et * SQ + stl * 128 : et * SQ + (stl + 1) * 128],
                            rhs=wv_cs[ci][:, le * DL : (le + 1) * DL],
                            start=(et == 0),
                            stop=(et == NET - 1),
                        )
                    store_v(psv[:, :DL], st)

                def emit_qk(sl, w_sb, base4, bias_base, dt, on_dve=False):
                    # transposed [d, s] projection for one head. Bias add on
                    # ACT normally; interleaved K units use DVE so they do
                    # not delay the exp stream queued on ACT.
                    xt_e = xt_tiles[sl]
                    psq = pa.tile([128, SQ], F32, name="psq", tag="pa")
                    for et in range(NET):
                        nc.tensor.matmul(
                            psq[:, :],
                            lhsT=w_sb[:, et * DL + dt * 128 : et * DL + (dt + 1) * 128],
                            rhs=xt_e[:, et * SQ : (et + 1) * SQ],
                            start=(et == 0),
                            stop=(et == NET - 1),
                        )
                    dst = qkt[:, (base4 + dt) * S + sl * SQ : (base4 + dt) * S + (sl + 1) * SQ]
                    if on_dve:
                        nc.vector.tensor_scalar_add(
                            out=dst, in0=psq[:, :],
                            scalar1=biasqk[:, bias_base + dt : bias_base + dt + 1],
                        )
                    else:
                        nc.scalar.add(
                            dst, psq[:, :],
                            biasqk[:, bias_base + dt : bias_base + dt + 1],
                        )

                def vslice(kt, h):
                    return vt[:, kt * VW + h * 129 : kt * VW + (h + 1) * 129]

                def emit_norm_chain(psO, qb, h, qc, on_act):
                    # 1/denom (col 128) times the value cols, then a PE
                    # transpose back to [d, q] for the O-projection
                    recip = rp.tile([128, 1], F32, name="recip", tag="recip")
                    nc.vector.reciprocal_approx_fast(
                        recip[:, :], psO[h, qc][:, 128:129]
                    )
                    stg = rp.tile([128, 128], BF16, name="stg", tag="stg")
                    nc.vector.tensor_scalar_mul(
                        out=stg[:, :], in0=psO[h, qc][:, 0:128],
                        scalar1=recip[:, :],
                    )
                    psT = pa.tile([128, 128], BF16, name="psT", tag="pa")
                    nc.tensor.transpose(psT[:, :], stg[:, :], idt_sb[:, :])
                    dst = outt[:, (h * NQB + qb) * QB + qc * 128 :
                               (h * NQB + qb) * QB + (qc + 1) * 128]
                    if on_act:
                        nc.scalar.copy(dst, psT[:, :])
                    else:
                        nc.vector.tensor_copy(dst, psT[:, :])

                def o_proj(qb, tail=False):
                    for j in range(2):
                        st = qb * 2 + j
                        osb = op.tile([128, D], BF16, name="osb", tag="osb")
                        for et in range(4):
                            psF = pa.tile([128, 512], F32, name="psF", tag="pa")
                            for h in range(NH):
                                o_base = (h * NQB + qb) * QB + j * 128
                                nc.tensor.matmul(
                                    psF[:, :],
                                    lhsT=outt[:, o_base : o_base + 128],
                                    rhs=wot_sb[:, h * D + et * 512 : h * D + (et + 1) * 512],
                                    start=(h == 0),
                                    stop=(h == NH - 1),
                                )
                            if tail and et % 2 == 1:
                                # tail: ACT is idle — split the drain copies
                                nc.scalar.copy(
                                    osb[:, et * 512 : (et + 1) * 512], psF[:, :]
                                )
                            else:
                                nc.vector.tensor_copy(
                                    osb[:, et * 512 : (et + 1) * 512], psF[:, :]
                                )
                            if tail and et == 1:
                                nc.sync.dma_start(
                                    out=OUT[st * 128 : (st + 1) * 128, :1024],
                                    in_=osb[:, :1024],
                                )
                        if tail:
                            nc.sync.dma_start(
                                out=OUT[st * 128 : (st + 1) * 128, 1024:],
                                in_=osb[:, 1024:],
                            )
                        else:
                            nc.sync.dma_start(
                                out=OUT[st * 128 : (st + 1) * 128, :], in_=osb[:, :]
                            )

                prev_block = [None]  # (psO dict, qb) of the not-yet-drained block

                def finish_prev(tail=False):
                    # normalize+transpose and O-project the PREVIOUS block.
                    # Deferred into the NEXT block (after its first scores)
                    # so PE rolls from this block's diagonal AV straight into
                    # the next block's scores with no normalize bubble.
                    if prev_block[0] is None:
                        return
                    ppsO, pqb = prev_block[0]
                    prev_block[0] = None
                    for h in range(NH):
                        for qc in range(2):
                            emit_norm_chain(ppsO, pqb, h, qc, on_act=False)
                    o_proj(pqb, tail=tail)

                def attention_qb(qb, units=None, diag_units=None):
                    # Software-pipelined: AV matmuls run one pair behind the
                    # scores/exp stream (so PE never waits on the exp it just
                    # queued). V units of this slice land between the
                    # diagonal's scores and its AV (which needs them).
                    npairs = (qb + 1) if is_causal else NQB
                    units = list(units) if units else []
                    diag_units = list(diag_units) if diag_units else []
                    per_gap = -(-len(units) // max(1, npairs - 1)) if units else 0
                    psO = {}

                    def alloc_psO():
                        for h in range(NH):
                            for qc in range(2):
                                psO[h, qc] = pso.tile(
                                    [128, 512], F32, name="psO", tag="o"
                                )

                    def emit_scores(pi, h):
                        qb0 = h * S + qb * QB
                        psS = scor.tile([128, SQ], F32, name="psS", tag="sc")
                        p2 = pp.tile([128, SQ], BF16, name="p2", tag="p")
                        if is_causal and pi == qb:
                            # diagonal pair: tile i=0 spans the full 256
                            # (masked), tile i=1 only q-cols 128:256; one exp
                            kt0, kt1 = 2 * qb, 2 * qb + 1
                            nc.tensor.matmul(
                                psS[:, 0:QB],
                                lhsT=qkt[:, (2 + h) * S + kt0 * 128 : (2 + h) * S + (kt0 + 1) * 128],
                                rhs=qkt[:, qb0 : qb0 + QB],
                                start=True,
                                stop=True,
                            )
                            nc.tensor.matmul(
                                psS[:, QB : QB + 128],
                                lhsT=qkt[:, (2 + h) * S + kt1 * 128 : (2 + h) * S + (kt1 + 1) * 128],
                                rhs=qkt[:, qb0 + 128 : qb0 + QB],
                                start=True,
                                stop=True,
                            )
                            nc.scalar.activation(
                                p2[:, 0 : QB + 128], psS[:, 0 : QB + 128],
                                mybir.ActivationFunctionType.Exp,
                                scale=float(SCALE),
                            )
                            nc.vector.tensor_mul(
                                p2[:, 0:QB], p2[:, 0:QB], masks_sb[:, 0:QB]
                            )
                            nc.vector.tensor_mul(
                                p2[:, QB : QB + 128], p2[:, QB : QB + 128],
                                masks_sb[:, 0:128],
                            )
                        else:
                            for half in range(2):
                                kt = 2 * pi + half
                                nc.tensor.matmul(
                                    psS[:, half * QB : (half + 1) * QB],
                                    lhsT=qkt[:, (2 + h) * S + kt * 128 : (2 + h) * S + (kt + 1) * 128],
                                    rhs=qkt[:, qb0 : qb0 + QB],
                                    start=True,
                                    stop=True,
                                )
                            nc.scalar.activation(
                                p2[:, :], psS[:, :],
                                mybir.ActivationFunctionType.Exp,
                                scale=float(SCALE),
                            )
                        return p2

                    def emit_av(pi, h, p2):
                        if is_causal and pi == qb:
                            kt0, kt1 = 2 * qb, 2 * qb + 1
                            first = qb == 0
                            nc.tensor.matmul(
                                psO[h, 0][:, :129],
                                lhsT=p2[:, 0:128],
                                rhs=vslice(kt0, h),
                                start=first,
                                stop=True,
                            )
                            nc.tensor.matmul(
                                psO[h, 1][:, :129],
                                lhsT=p2[:, 128:256],
                                rhs=vslice(kt0, h),
                                start=first,
                                stop=False,
                            )
                            nc.tensor.matmul(
                                psO[h, 1][:, :129],
                                lhsT=p2[:, QB : QB + 128],
                                rhs=vslice(kt1, h),
                                start=False,
                                stop=True,
                            )
                        else:
                            last = (not is_causal) and pi == npairs - 1
                            for half in range(2):
                                kt = 2 * pi + half
                                for qc in range(2):
                                    nc.tensor.matmul(
                                        psO[h, qc][:, :129],
                                        lhsT=p2[:, half * QB + qc * 128 : half * QB + (qc + 1) * 128],
                                        rhs=vslice(kt, h),
                                        start=(pi == 0 and half == 0),
                                        stop=(last and half == 1),
                                    )

                    prev = None  # (pi, p2_h0, p2_h1)
                    for pi in range(npairs):
                        diag = is_causal and pi == qb
                        if units and diag:
                            while units:
                                units.pop(0)()
                        elif units and pi > 0:
                            for _ in range(per_gap):
                                if units:
                                    units.pop(0)()
                        p2s = [emit_scores(pi, h) for h in range(NH)]
                        if pi == 0:
                            finish_prev()
                            alloc_psO()
                        if prev is not None:
                            for h in range(NH):
                                emit_av(prev[0], h, prev[1 + h])
                            prev = None
                        if diag:
                            while diag_units:
                                diag_units.pop(0)()
                            for h in range(NH):
                                emit_av(pi, h, p2s[h])
                            prev_block[0] = (psO, qb)
                        else:
                            prev = (pi, p2s[0], p2s[1])
                    if prev is not None:  # non-causal: drain last pair
                        for h in range(NH):
                            emit_av(prev[0], h, prev[1 + h])
                    if not is_causal:
                        for h in range(NH):
                            for qc in range(2):
                                emit_norm_chain(psO, qb, h, qc, on_act=False)
                        o_proj(qb)

                if is_causal:
                    # Q of slice sl+1 is emitted as gap-fill units inside
                    # block 2sl+1 (always before block 2sl+2 needs it)
                    def q_units(sl):
                        if sl >= NSQ:
                            return []
                        return [
                            (lambda s=sl, d=d: emit_qk(s, wq_sb, 0, 0, d, on_dve=True))
                            for d in range(NH)
                        ]
                    for sl in range(NSQ):
                        if sl == 0:
                            emit_v0_all()
                            for d in range(NH):
                                emit_qk0(wk_sb, 2, 2, d)
                            for d in range(NH):
                                emit_qk0(wq_sb, 0, 0, d)
                            # slice-1 X load AFTER the startup burst so its
                            # 2MB transfer does not delay the weight DMAs
                            load_xe(1)
                            attention_qb(0, [])
                            attention_qb(1, q_units(1))
                            continue
                        if sl + 1 < NSQ:
                            load_xe(sl + 1)
                        units_k = [
                            (lambda s=sl, d=d: emit_qk(s, wk_sb, 2, 2, d, on_dve=True))
                            for d in range(NH)
                        ]
                        diag_a = [
                            (lambda s=sl, j=j: emit_v_tile(s, j)) for j in (0, 1)
                        ]
                        diag_b = [
                            (lambda s=sl, j=j: emit_v_tile(s, j)) for j in (2, 3)
                        ]
                        attention_qb(2 * sl, units_k, diag_a)
                        attention_qb(2 * sl + 1, q_units(sl + 1), diag_b)
                    finish_prev(tail=True)
                else:
                    for sl in range(NSQ):
                        if sl + 1 < NSQ:
                            load_xe(sl + 1)
                        if sl == 0:
                            emit_v0_all()
                            for d in range(NH):
                                emit_qk0(wk_sb, 2, 2, d)
                            for d in range(NH):
                                emit_qk0(wq_sb, 0, 0, d)
                            continue
                        for j in range(SQ // 128):
                            emit_v_tile(sl, j)
                        for w_sb, base4, bias_base in ((wq_sb, 0, 0), (wk_sb, 2, 2)):
                            for dt in range(NH):
                                emit_qk(sl, w_sb, base4, bias_base, dt)
                    for qb in range(NQB):
                        attention_qb(qb)
    nc.finalize()
    return nc


def _bf16(a: np.ndarray) -> np.ndarray:
    return np.ascontiguousarray(a.astype(ml_dtypes.bfloat16))


def make_in_maps(X, Wq, bq, Wk, bk, Wv, bv, Wo, is_causal: bool):
    x2d = np.asarray(X, dtype=np.float32).reshape(S, D)
    # xt2[sl*128+p, et*512+c] = X^T[et*128+p, sl*512+c]
    xt2 = _bf16(
        x2d.T.reshape(NET, 128, NSQ, SQ)
        .transpose(2, 1, 0, 3)
        .reshape(NSQ * 128, NET * SQ)
    )
    ki = np.arange(128)[:, None]
    qj = np.arange(QB)[None, :]
    masks = (ki <= qj).astype(ml_dtypes.bfloat16)
    idt = np.eye(128, dtype=ml_dtypes.bfloat16)

    def _pack_w(wT):  # [D, DL] -> [128, NET*DL]
        return _bf16(
            np.ascontiguousarray(wT).reshape(NET, 128, DL)
            .transpose(1, 0, 2)
            .reshape(128, NET * DL)
        )

    in_maps = []
    for c in range(NCORES):
        sl = slice(c * DL, (c + 1) * DL)
        wot = np.asarray(Wo)[:, sl].T  # [DL, D]
        wo2 = _bf16(wot.reshape(NH, 128, D).transpose(1, 0, 2).reshape(128, NH * D))
        in_maps.append(
            {
                "xt2": xt2,
                "wq2": _pack_w(np.asarray(Wq)[sl, :].T),
                "wk2": _pack_w(np.asarray(Wk)[sl, :].T),
                "wv2": _pack_w(np.asarray(Wv)[sl, :].T),
                "bqkc": np.ascontiguousarray(
                    np.stack(
                        [
                            np.asarray(bq, dtype=np.float32)[sl][:128],
                            np.asarray(bq, dtype=np.float32)[sl][128:],
                            np.asarray(bk, dtype=np.float32)[sl][:128],
                            np.asarray(bk, dtype=np.float32)[sl][128:],
                        ],
                        axis=1,
                    )
                ),
                "bvrow": _bf16(np.asarray(bv)[None, sl]),
                "wo2": wo2,
                "masks2": masks,
                "idt": idt,
            }
        )
    return in_maps


_NC_CACHE: dict = {}


def _get_nc(is_causal: bool) -> bass.Bass:
    if is_causal not in _NC_CACHE:
        _NC_CACHE[is_causal] = build_nc(is_causal)
    return _NC_CACHE[is_causal]


def kernel(X, Wq, bq, Wk, bk, Wv, bv, Wo, bo, is_causal, **run_kwargs):
    causal = bool(int(np.asarray(is_causal)))
    nc = _get_nc(causal)
    in_maps = make_in_maps(X, Wq, bq, Wk, bk, Wv, bv, Wo, causal)
    res = run_bass_kernel_spmd(nc, in_maps, core_ids=list(range(NCORES)), **run_kwargs)
    out = np.asarray(bo, dtype=np.float32)[None, :].repeat(S, axis=0)
    for c in range(NCORES):
        out += np.asarray(res.results[c]["out"], dtype=np.float32)
    return out.reshape(1, S, D)
